# revision 1
# baseline (speedup 1.0000x reference)
"""Multi-head attention forward on 8 Trainium2 NeuronCores.

Problem: x[4,2048,1024], 16 heads (d=64), fp32. out = softmax(QK^T/sqrt(d) + mask) V @ Wo.

Sharding: core = (batch b in 0..3) x (head-group hg in 0..1). Each core handles one
batch element and 8 heads (a 512-wide slice of the model dim). Each core emits a
partial output [2048,1024] (its heads' contribution through Wo); the host sums the
two head-group partials per batch element.

All matmuls run in float32r (full-rate fp32 on the PE for moving dim >= 256). The
BIR verifier requires fp32r matmul operands to be *typed* fp32r at their producing
instruction, so every producer (DMA loads, DVE copies, ACT exp) writes through a
bitcast AP.

Per-core pipeline:
  A) QT,KT = projections in transposed layout [512,2048] (heads pair-packed along
     partitions); V in natural layout, head-interleaved with a ones column per head
     ([128, 8*65]) so the attention matmul also produces the softmax row sums.
  B) per head-pair (2m, 2m+1), per 512-query tile, per 128-key chunk: two K=64
     logits matmuls land in the two halves of a [128,1024] PSUM tile (the two
     heads sit in PE row-groups 0-1/2-3 and run concurrently); one ACT exp over
     [128,1024] with the key mask as per-partition bias and 1/sqrt(d) folded into
     the activation scale; two PT @ V_aug accumulations -> attn_aug[65,512] per
     head (row 64 = exp row sum). Then per head: DVE reciprocal of the row-sum
     row, K=1 outer-product matmul broadcasts it over 64 partitions, DVE multiply
     writes normalized attn^T into SBUF.
  C) out[q,:] = attnT-chunk^T @ Wo-slice (K=512 accumulation), DVE copy, DMA out.
"""
import sys

sys.path.insert(0, "/opt/trn_rl_repo")

import numpy as np

import concourse.bass as bass
import concourse.tile as tile
from concourse import mybir
from concourse.bass_utils import run_bass_kernel_spmd
from concourse.vector_clock import ScopedClock

_wsplit_ctr = [0]


def split_multi_waits(nc):
    """The walrus build in this container accepts at most ONE sync wait per
    instruction. Split any instruction carrying N>1 waits into (N-1)
    single-wait nops on the same engine immediately before it; the original
    instruction keeps one wait and all its updates."""
    for f in nc.m.functions:
        for bb in f.blocks:
            out = []
            changed = False
            for inst in bb.instructions:
                si = inst.sync_info
                waits = list(si.on_wait) if si is not None and si.on_wait else []
                if len(waits) > 1:
                    updates = list(si.on_update) if si.on_update else []
                    for w in waits[1:]:
                        _wsplit_ctr[0] += 1
                        nop = mybir.InstNoOp(
                            name=f"I-wsplit-{_wsplit_ctr[0]}", ins=[], outs=[]
                        )
                        nop.engine = inst.engine
                        nop.sync_info = mybir.SyncInfo(on_wait=[w], on_update=[])
                        out.append(nop)
                    inst.sync_info = mybir.SyncInfo(on_wait=[waits[0]], on_update=updates)
                    changed = True
                out.append(inst)
            if changed:
                bb.instructions = out
    return nc

B, S, D, H, DH = 4, 2048, 1024, 16, 64
HG = 2  # head groups (tensor-parallel)
LD = D // HG  # 512 local model-dim slice
LH = H // HG  # 8 local heads
N_CORES = B * HG
SCALE = float(DH) ** -0.5
NEG_INF = -1e30

FP = mybir.dt.float32
FPR = mybir.dt.float32r
BF = mybir.dt.bfloat16

KC = D // 128  # 8 contraction chunks (projections)
MC = LD // 128  # 4 row chunks of the local dim (= head pairs)
SC = S // 128  # 16 seq chunks of 128
QT = S // 512  # 4 query tiles of 512
Exp = mybir.ActivationFunctionType.Exp
E1 = DH + 1  # per-head V stride incl. ones column


def _fr(ap):
    return ap.bitcast(FPR)


class SplitDrainTileContext(tile.TileContext):
    """The walrus build in this container rejects a Drain instruction with
    more than one sync wait; gate the tail drain with single-wait nops."""

    def _drain_and_barrier(self, tick_clock, wait_clock):
        nc = self.nc
        probe = nc.sync.nop()
        wait_clock.add_sem_waits(
            probe.ins, ScopedClock({None: tick_clock.global_clock})
        )
        si = probe.ins.sync_info
        waits = list(si.on_wait) if si is not None and si.on_wait else []
        updates = list(si.on_update) if si is not None and si.on_update else []
        if len(waits) > 1:
            probe.ins.sync_info = mybir.SyncInfo(on_wait=[waits[0]], on_update=updates)
            for w in waits[1:]:
                n2 = nc.sync.nop()
                n2.ins.sync_info = mybir.SyncInfo(on_wait=[w], on_update=[])
        nc.sync.drain()
        nc.all_engine_barrier()
        popped = nc._tile_sem_poison_stack.pop()
        assert popped is self._sem_poison
        nc.clear_and_free_semaphores(list(self.sems.allocated().values()))
        nc.all_engine_barrier()


def build_nc(for_hw=True):
    nc = bass.Bass(trn_type="TRN2")
    xT = nc.dram_tensor("xT", [D, S], FP, kind="ExternalInput").ap()
    wq = nc.dram_tensor("wq", [D, LD], FP, kind="ExternalInput").ap()
    wk = nc.dram_tensor("wk", [D, LD], FP, kind="ExternalInput").ap()
    wv = nc.dram_tensor("wv", [D, LD], FP, kind="ExternalInput").ap()
    wo = nc.dram_tensor("wo", [LD, D], FP, kind="ExternalInput").ap()
    kbias = nc.dram_tensor("kbias", [128, SC], FP, kind="ExternalInput").ap()
    ones_d = nc.dram_tensor("ones_d", [1, 64], FP, kind="ExternalInput").ap()
    ones_b = nc.dram_tensor("ones_b", [128, LH], BF, kind="ExternalInput").ap()
    out = nc.dram_tensor("out", [S, D], FP, kind="ExternalOutput").ap()

    with SplitDrainTileContext(nc) as tc:
        _body(tc, xT, wq, wk, wv, wo, kbias, ones_d, ones_b, out)
    if for_hw:
        split_multi_waits(nc)
    return nc


def _body(tc, xT, wq, wk, wv, wo, kbias, ones_d, ones_b, out):
    nc = tc.nc
    with (
        tc.tile_pool(name="pers", bufs=1) as pers,
        tc.tile_pool(name="pt", bufs=3) as pt_pool,
        tc.tile_pool(name="rs", bufs=2) as rs_pool,
        tc.tile_pool(name="ot", bufs=4) as ot_pool,
        tc.tile_pool(name="psmm", bufs=1, space="PSUM") as psmm,
    ):
        qt = [pers.tile([128, S], FP, tag=f"qt{m}", name=f"qt{m}") for m in range(MC)]
        kt = [pers.tile([128, S], FP, tag=f"kt{m}", name=f"kt{m}") for m in range(MC)]
        vt = [pers.tile([128, LH * E1], BF, tag=f"v{s}", name=f"v{s}") for s in range(SC)]
        att = [pers.tile([128, S], FP, tag=f"at{m}", name=f"at{m}") for m in range(MC)]
        biasT = pers.tile([128, SC], FP, tag="biasT")
        ones64 = pers.tile([1, 64], FP, tag="ones64")

        nc.sync.dma_start(biasT[:], kbias[:])
        nc.sync.dma_start(_fr(ones64[:]), _fr(ones_d[:]))
        for s in range(SC):
            # fill each head's ones column of V_aug straight from DRAM
            dst = vt[s][:].rearrange("p (h e) -> p h e", e=E1)[:, :, DH : DH + 1]
            nc.sync.dma_start(dst, ones_b[:, 0:LH].unsqueeze(2))

        # ---- stage A: projections ----
        with (
            tc.tile_pool(name="xt", bufs=1) as xt_pool,
            tc.tile_pool(name="w", bufs=1) as w_pool,
        ):
            def load_w(wdram, cast=True):
                wts = [
                    w_pool.tile([128, LD], FP, tag=f"w{k}", name=f"w{k}")
                    for k in range(KC)
                ]
                for k in range(KC):
                    src = wdram[k * 128 : (k + 1) * 128, :]
                    nc.sync.dma_start(_fr(wts[k][:]), _fr(src))
                return wts

            def v_pass(xts, half, wts):
                for sc in range(SC // 2):
                    s_idx = half * (SC // 2) + sc
                    ps = psmm.tile([128, 512], FP, tag="ps", name="ps", bufs=2)
                    j, off = sc // 4, (sc % 4) * 128
                    for k in range(KC):
                        nc.tensor.matmul(
                            ps[:],
                            _fr(xts[k][j][:, off : off + 128]),
                            _fr(wts[k][:]),
                            start=(k == 0),
                            stop=(k == KC - 1),
                        )
                    src = ps[:].rearrange("p (h e) -> p h e", h=LH)
                    dst = vt[s_idx][:].rearrange("p (h e) -> p h e", e=E1)[:, :, 0:DH]
                    nc.vector.tensor_copy(dst, src)

            def qk_pass(xts, half, wts, dstT, ms):
                for m in ms:
                    for q2 in range(2):
                        ps = psmm.tile([128, 512], FP, tag="ps", name="ps", bufs=2)
                        for k in range(KC):
                            nc.tensor.matmul(
                                ps[:],
                                _fr(wts[k][:, m * 128 : (m + 1) * 128]),
                                _fr(xts[k][q2][:]),
                                start=(k == 0),
                                stop=(k == KC - 1),
                            )
                        qlo = half * 1024 + q2 * 512
                        nc.vector.tensor_copy(_fr(dstT[m][:, qlo : qlo + 512]), ps[:])

            for half in range(2):
                # two 512-col slices per contraction chunk: the first V matmul
                # needs only the j=0 slices (2MB) instead of the full half (4MB)
                xts = [
                    [
                        xt_pool.tile([128, 512], FP, tag=f"xt{k}_{j}", name=f"xt{k}_{j}")
                        for j in range(2)
                    ]
                    for k in range(KC)
                ]
                for j in range(2):
                    for k in range(KC):
                        lo = half * 1024 + j * 512
                        nc.sync.dma_start(
                            _fr(xts[k][j][:]),
                            _fr(xT[k * 128 : (k + 1) * 128, lo : lo + 512]),
                        )
                if half == 0:
                    # V first (stage B's AV loop hits half-1 V chunks first)
                    v_pass(xts, half, load_w(wv))
                    qk_pass(xts, half, load_w(wq), qt, range(MC))
                    qk_pass(xts, half, load_w(wk), kt, range(MC))
                else:
                    # finish pair m=0 first so stage B starts while A finishes
                    wq_t = load_w(wq)
                    qk_pass(xts, half, wq_t, qt, [0])
                    wk_t = load_w(wk)
                    qk_pass(xts, half, wk_t, kt, [0])
                    v_pass(xts, half, load_w(wv))
                    qk_pass(xts, half, load_w(wq), qt, [1, 2, 3])
                    qk_pass(xts, half, load_w(wk), kt, [1, 2, 3])

        # ---- stages B+C ----
        with tc.tile_pool(name="wo", bufs=1) as wo_pool:
            wos = [
                wo_pool.tile([128, D], FP, tag=f"wo{j}", name=f"wo{j}")
                for j in range(MC)
            ]
            for j in range(MC):
                nc.sync.dma_start(_fr(wos[j][:]), _fr(wo[j * 128 : (j + 1) * 128, :]))

            def stage_c_slab(q):
                # output projection for one 512-query slab (4 chunks of 128)
                for qc in range(4 * q, 4 * (q + 1)):
                    for n in range(2):
                        ps = psmm.tile([128, 512], FP, tag="ps", name="psc", bufs=2)
                        for j in range(MC):
                            nc.tensor.matmul(
                                ps[:],
                                _fr(att[j][:, qc * 128 : (qc + 1) * 128]),
                                _fr(wos[j][:, n * 512 : (n + 1) * 512]),
                                start=(j == 0),
                                stop=(j == MC - 1),
                            )
                        ot = ot_pool.tile([128, 512], FP, tag="ot", name="ot")
                        nc.vector.tensor_copy(ot[:], ps[:])
                        nc.sync.dma_start(
                            out[qc * 128 : (qc + 1) * 128, n * 512 : (n + 1) * 512],
                            ot[:],
                        )

            # stage B: attention, one head-pair at a time
            for m in range(MC):
                hA, hB = 2 * m, 2 * m + 1
                for q in range(QT):
                    qs = slice(q * 512, (q + 1) * 512)
                    aA = psmm.tile([128, 512], FP, tag="aA", name="aA")
                    aB = psmm.tile([128, 512], FP, tag="aB", name="aB")
                    for kc in range(SC):
                        ks = slice(kc * 128, (kc + 1) * 128)
                        lg = psmm.tile([128, 1024], FP, tag="lg", name="lg", bufs=2)
                        nc.tensor.matmul(
                            lg[:, 0:512],
                            _fr(kt[m][0:64, ks]),
                            _fr(qt[m][0:64, qs]),
                            start=True,
                            stop=True,
                        )
                        nc.tensor.matmul(
                            lg[:, 512:1024],
                            _fr(kt[m][64:128, ks]),
                            _fr(qt[m][64:128, qs]),
                            start=True,
                            stop=True,
                        )
                        pt = pt_pool.tile([128, 1024], BF, tag="pt", name="pt")
                        nc.scalar.activation(
                            pt[:], lg[:], Exp, bias=biasT[:, kc : kc + 1], scale=SCALE
                        )
                        nc.tensor.matmul(
                            aA[0:65, :],
                            vt[kc][:, hA * E1 : (hA + 1) * E1],
                            pt[:, 0:512],
                            start=(kc == 0),
                            stop=(kc == SC - 1),
                            skip_group_check=True,
                        )
                        nc.tensor.matmul(
                            aB[0:65, :],
                            vt[kc][:, hB * E1 : (hB + 1) * E1],
                            pt[:, 512:1024],
                            start=(kc == 0),
                            stop=(kc == SC - 1),
                            skip_group_check=True,
                        )
                    for po, a_ps in ((0, aA), (64, aB)):
                        rs = rs_pool.tile([1, 512], FP, tag="rs", name="rs")
                        with nc.allow_low_precision(reason="fp32r operand typing"):
                            nc.vector.reciprocal(_fr(rs[:]), a_ps[64:65, :])
                        bc = psmm.tile([64, 512], FP, tag="ps", name="bc", bufs=2)
                        nc.tensor.matmul(
                            bc[:], _fr(ones64[:]), _fr(rs[:]), start=True, stop=True
                        )
                        bcs = rs_pool.tile([64, 512], FP, tag="bcs", name="bcs", bufs=2)
                        nc.vector.tensor_copy(bcs[:], bc[:])
                        nc.vector.tensor_tensor(
                            out=_fr(att[m][po : po + 64, qs]),
                            in0=a_ps[0:64, :],
                            in1=bcs[:],
                            op=mybir.AluOpType.mult,
                        )
                    if m == MC - 1:
                        stage_c_slab(q)



_nc = None


def get_nc():
    global _nc
    if _nc is None:
        _nc = build_nc()
    return _nc


def make_in_maps(x, mask, Wq, Wk, Wv, Wo):
    x = np.asarray(x, dtype=np.float32)
    mask = np.asarray(mask)
    Wq, Wk, Wv, Wo = (np.asarray(w, dtype=np.float32) for w in (Wq, Wk, Wv, Wo))
    in_maps = []
    for c in range(N_CORES):
        b, hg = c // HG, c % HG
        lo, hi = hg * LD, (hg + 1) * LD
        kb = np.where(mask[b], 0.0, NEG_INF).astype(np.float32)
        in_maps.append(
            {
                "xT": np.ascontiguousarray(x[b].T),
                "wq": np.ascontiguousarray(Wq[:, lo:hi]),
                "wk": np.ascontiguousarray(Wk[:, lo:hi]),
                "wv": np.ascontiguousarray(Wv[:, lo:hi]),
                "wo": np.ascontiguousarray(Wo[lo:hi, :]),
                "kbias": np.ascontiguousarray(kb.reshape(SC, 128).T),
                "ones_d": np.ones((1, 64), np.float32),
                "ones_b": np.ones((128, LH), np.float32).astype(__import__("ml_dtypes").bfloat16),
            }
        )
    return in_maps


def kernel(x, mask, Wq, Wk, Wv, Wo):
    nc = get_nc()
    in_maps = make_in_maps(x, mask, Wq, Wk, Wv, Wo)
    res = run_bass_kernel_spmd(nc, in_maps, list(range(N_CORES)))
    outs = np.empty((B, S, D), dtype=np.float32)
    for b in range(B):
        outs[b] = res.results[2 * b]["out"] + res.results[2 * b + 1]["out"]
    return outs



# revision 2
# speedup vs baseline: 1.3385x; 1.3385x over previous
"""Multi-head attention forward on 8 Trainium2 NeuronCores — v3.

Like v2 (AV in [q,d] orientation, bf16 Q/K/attn/Wo, per-partition normalize,
PE transposes) but with pipeline-aware emission for the in-order engines:

- Within a phase (head pair m, query tile qt), logits+exp for key chunk kc+2 are
  emitted BEFORE the AV matmuls of chunk kc, so the ACT engine always has two
  exp instructions in flight when an AV matmul blocks in the PE wait queue.
- The attn transposes of phase i are deferred into phase i+1 (through the cps
  PSUM ring, not the lg ring), so they never delay the next phase's logits.
- Stage-A projection chunks and stage-C output-projection groups drip one per
  key chunk through the cps ring, placed after the logits emission point.
- PSUM: lg [128,1024]x2 (4 banks), avA/avB [128,260]x1 (2), cps [128,512]x2 (2).
"""
import sys

sys.path.insert(0, "/opt/trn_rl_repo")

import numpy as np
from collections import deque

import concourse.bass as bass
import concourse.tile as tile
from concourse import mybir
from concourse.bass_utils import run_bass_kernel_spmd
from concourse.vector_clock import ScopedClock

_wsplit_ctr = [0]


def split_multi_waits(nc):
    """Walrus accepts at most one sync wait per instruction; split extras
    into single-wait nops."""
    for f in nc.m.functions:
        for bb in f.blocks:
            out = []
            changed = False
            for inst in bb.instructions:
                si = inst.sync_info
                waits = list(si.on_wait) if si is not None and si.on_wait else []
                if len(waits) > 1:
                    updates = list(si.on_update) if si.on_update else []
                    for w in waits[1:]:
                        _wsplit_ctr[0] += 1
                        nop = mybir.InstNoOp(
                            name=f"I-wsplit-{_wsplit_ctr[0]}", ins=[], outs=[]
                        )
                        nop.engine = inst.engine
                        nop.sync_info = mybir.SyncInfo(on_wait=[w], on_update=[])
                        out.append(nop)
                    inst.sync_info = mybir.SyncInfo(on_wait=[waits[0]], on_update=updates)
                    changed = True
                out.append(inst)
            if changed:
                bb.instructions = out
    return nc


B, S, D, H, DH = 4, 2048, 1024, 16, 64
HG = 2
LD = D // HG
LH = H // HG
N_CORES = B * HG
SCALE = float(DH) ** -0.5
NEG_INF = -1e30

FP = mybir.dt.float32
FPR = mybir.dt.float32r
BF = mybir.dt.bfloat16

KC = D // 128
MC = LD // 128
SC = S // 128
QT = S // 512
Exp = mybir.ActivationFunctionType.Exp
E1 = DH + 1


def _fr(ap):
    return ap.bitcast(FPR)


class SplitDrainTileContext(tile.TileContext):
    def _drain_and_barrier(self, tick_clock, wait_clock):
        nc = self.nc
        probe = nc.sync.nop()
        wait_clock.add_sem_waits(
            probe.ins, ScopedClock({None: tick_clock.global_clock})
        )
        si = probe.ins.sync_info
        waits = list(si.on_wait) if si is not None and si.on_wait else []
        updates = list(si.on_update) if si is not None and si.on_update else []
        if len(waits) > 1:
            probe.ins.sync_info = mybir.SyncInfo(on_wait=[waits[0]], on_update=updates)
            for w in waits[1:]:
                n2 = nc.sync.nop()
                n2.ins.sync_info = mybir.SyncInfo(on_wait=[w], on_update=[])
        nc.sync.drain()
        nc.all_engine_barrier()
        popped = nc._tile_sem_poison_stack.pop()
        assert popped is self._sem_poison
        nc.clear_and_free_semaphores(list(self.sems.allocated().values()))
        nc.all_engine_barrier()


def build_nc(for_hw=True):
    nc = bass.Bass(trn_type="TRN2")
    xT = nc.dram_tensor("xT", [D, S], FP, kind="ExternalInput").ap()
    wq = nc.dram_tensor("wq", [D, LD], FP, kind="ExternalInput").ap()
    wk = nc.dram_tensor("wk", [D, LD], FP, kind="ExternalInput").ap()
    wv = nc.dram_tensor("wv", [D, LD], FP, kind="ExternalInput").ap()
    wo = nc.dram_tensor("wo", [LD, D], BF, kind="ExternalInput").ap()
    kbias = nc.dram_tensor("kbias", [128, SC], FP, kind="ExternalInput").ap()
    idn = nc.dram_tensor("idn", [128, 128], BF, kind="ExternalInput").ap()
    out = nc.dram_tensor("out", [S, D], FP, kind="ExternalOutput").ap()

    with SplitDrainTileContext(nc) as tc:
        _body(tc, xT, wq, wk, wv, wo, kbias, idn, out)
    if for_hw:
        split_multi_waits(nc)
    return nc


def _body(tc, xT, wq, wk, wv, wo, kbias, idn, out):
    nc = tc.nc
    with (
        tc.tile_pool(name="pers", bufs=1) as pers,
        tc.tile_pool(name="pt", bufs=4) as pt_pool,
        tc.tile_pool(name="a2", bufs=8) as a2_pool,
        tc.tile_pool(name="rs", bufs=4) as rs_pool,
        tc.tile_pool(name="ot", bufs=3) as ot_pool,
        tc.tile_pool(name="psmm", bufs=1, space="PSUM") as psum,
    ):
        qt = [pers.tile([128, S], BF, tag=f"qt{m}", name=f"qt{m}") for m in range(MC)]
        kt = [pers.tile([128, S], BF, tag=f"kt{m}", name=f"kt{m}") for m in range(MC)]
        vt = [pers.tile([128, LH * E1], BF, tag=f"v{s}", name=f"v{s}") for s in range(SC)]
        attT = [pers.tile([128, S], BF, tag=f"at{m}", name=f"at{m}") for m in range(MC)]
        biasT = pers.tile([128, SC], FP, tag="biasT")
        idn_sb = pers.tile([128, 128], BF, tag="idn")

        for s in range(SC):
            dst = vt[s][:].rearrange("p (h e) -> p h e", e=E1)[:, :, DH : DH + 1]
            nc.vector.memset(dst, 1.0)

        # ---------- stage B phase emitter ----------
        def emit_B(m, qti, drip, rate=2):
            """One attention phase. Emits logits/exp two key-chunks ahead of
            the AV matmuls; pops drip closures after the logits point of
            every `rate`-th key chunk."""
            hA, hB = 2 * m, 2 * m + 1
            qs = slice(qti * 512, (qti + 1) * 512)
            avA = psum.tile([128, 4 * E1], FP, tag="avA", name="avA", bufs=1)
            avB = psum.tile([128, 4 * E1], FP, tag="avB", name="avB", bufs=1)
            pts = {}

            def logits(kc):
                ks = slice(kc * 128, (kc + 1) * 128)
                lg = psum.tile([128, 1024], FP, tag="lg", name="lg", bufs=2)
                nc.tensor.matmul(
                    lg[:, 0:512], kt[m][0:64, ks], qt[m][0:64, qs],
                    start=True, stop=True,
                )
                nc.tensor.matmul(
                    lg[:, 512:1024], kt[m][64:128, ks], qt[m][64:128, qs],
                    start=True, stop=True,
                )
                pt = pt_pool.tile([128, 1024], BF, tag="pt", name="pt")
                nc.scalar.activation(
                    pt[:], lg[:], Exp, bias=biasT[:, kc : kc + 1], scale=SCALE
                )
                pts[kc] = pt

            logits(0)
            logits(1)
            for kc in range(SC):
                pt = pts.pop(kc)
                for qc in range(4):
                    # start=True only on the first slice: the PSUM zero-region
                    # "pending zero" marking spans the whole bank, so later
                    # start=True calls would wipe sibling slices' first chunk
                    nc.tensor.matmul(
                        avA[:, qc * E1 : (qc + 1) * E1],
                        pt[:, qc * 128 : (qc + 1) * 128],
                        vt[kc][:, hA * E1 : (hA + 1) * E1],
                        start=(kc == 0 and qc == 0), stop=(kc == SC - 1),
                        skip_group_check=True,
                    )
                    nc.tensor.matmul(
                        avB[:, qc * E1 : (qc + 1) * E1],
                        pt[:, 512 + qc * 128 : 512 + (qc + 1) * 128],
                        vt[kc][:, hB * E1 : (hB + 1) * E1],
                        start=(kc == 0 and qc == 0), stop=(kc == SC - 1),
                        skip_group_check=True,
                    )
                if kc + 2 < SC:
                    logits(kc + 2)
                if drip and kc % rate == rate - 1:
                    drip.popleft()()
            while drip:
                drip.popleft()()
            # normalize into a2 staging tiles (bf16); transposes are deferred
            rsA = rs_pool.tile([128, 4], FP, tag="rs", name="rsA")
            rsB = rs_pool.tile([128, 4], FP, tag="rs", name="rsB")
            avAr = avA[:].rearrange("p (q e) -> p q e", e=E1)
            avBr = avB[:].rearrange("p (q e) -> p q e", e=E1)
            a2s = [a2_pool.tile([128, 128], BF, tag="a2", name="a2") for _ in range(4)]
            nc.vector.reciprocal(rsA[:], avAr[:, :, DH : DH + 1])
            for qc in range(4):
                nc.vector.tensor_scalar(
                    a2s[qc][:, 0:64], avA[:, qc * E1 : qc * E1 + DH],
                    rsA[:, qc : qc + 1], None, mybir.AluOpType.mult,
                )
            nc.vector.reciprocal(rsB[:], avBr[:, :, DH : DH + 1])
            for qc in range(4):
                nc.vector.tensor_scalar(
                    a2s[qc][:, 64:128], avB[:, qc * E1 : qc * E1 + DH],
                    rsB[:, qc : qc + 1], None, mybir.AluOpType.mult,
                )

            def tp_flush(m=m, qs=qs, a2s=a2s):
                tp = psum.tile([128, 512], BF, tag="cps", name="tp", bufs=2)
                for qc in range(4):
                    nc.tensor.matmul(
                        tp[:, qc * 128 : (qc + 1) * 128], a2s[qc][:], idn_sb[:],
                        is_transpose=True, skip_group_check=True,
                    )
                nc.vector.tensor_copy(attT[m][:, qs], tp[:])

            return tp_flush

        def c_group(qc, n, wo_all):
            def emit():
                cps = psum.tile([128, 512], FP, tag="cps", name="cps", bufs=2)
                for j in range(MC):
                    nc.tensor.matmul(
                        cps[:],
                        attT[j][:, qc * 128 : (qc + 1) * 128],
                        wo_all[:, j * D + n * 512 : j * D + (n + 1) * 512],
                        start=(j == 0), stop=(j == MC - 1),
                    )
                ot = ot_pool.tile([128, 512], FP, tag="ot", name="ot")
                nc.vector.tensor_copy(ot[:], cps[:])
                nc.sync.dma_start(
                    out[qc * 128 : (qc + 1) * 128, n * 512 : (n + 1) * 512],
                    ot[:],
                )

            return emit

        # ---------- stage A ----------
        with (
            tc.tile_pool(name="xt", bufs=1) as xt_pool,
            tc.tile_pool(name="w", bufs=1) as w_pool,
        ):
            # batched tiles: one DMA each (HWDGE gen is ~650ns per dma_start)
            wkm = [w_pool.tile([128, KC * 128], FP, tag=f"wkm{m}", name=f"wkm{m}") for m in range(MC)]
            wqm = [w_pool.tile([128, KC * 128], FP, tag=f"wqm{m}", name=f"wqm{m}") for m in range(MC)]
            wv_all = w_pool.tile([128, KC * LD], FP, tag="wv", name="wv_all")
            xs = [
                [xt_pool.tile([128, KC * 512], FP, tag=f"x{h}_{j}", name=f"x{h}_{j}") for j in range(2)]
                for h in range(2)
            ]
            wkR = wk.rearrange("(k r) c -> r k c", r=128)
            wqR = wq.rearrange("(k r) c -> r k c", r=128)
            wvR = wv.rearrange("(k r) c -> r k c", r=128)
            xR = xT.rearrange("(k r) q -> r k q", r=128)

            def xv(t):  # [128, (k c)] view
                return t[:].rearrange("r (k c) -> r k c", k=KC)

            # priority order: inputs of the head chunks first
            nc.sync.dma_start(_fr(xv(wkm[0])), _fr(wkR[:, :, 0:128]))
            nc.sync.dma_start(_fr(xv(xs[0][0])), _fr(xR[:, :, 0:512]))
            nc.sync.dma_start(_fr(xv(wqm[0])), _fr(wqR[:, :, 0:128]))
            nc.sync.dma_start(_fr(xv(wv_all)), _fr(wvR[:, :, :]))
            nc.sync.dma_start(biasT[:], kbias[:])
            nc.sync.dma_start(_fr(xv(xs[0][1])), _fr(xR[:, :, 512:1024]))
            for h, j in ((1, 0), (1, 1)):
                lo = h * 1024 + j * 512
                nc.sync.dma_start(_fr(xv(xs[h][j])), _fr(xR[:, :, lo : lo + 512]))
            for m in range(1, MC):
                nc.sync.dma_start(_fr(xv(wkm[m])), _fr(wkR[:, :, m * 128 : (m + 1) * 128]))
                nc.sync.dma_start(_fr(xv(wqm[m])), _fr(wqR[:, :, m * 128 : (m + 1) * 128]))
            nc.sync.dma_start(idn_sb[:], idn[:])

            def qk_chunk(half, wtm, dstT, m, q2):
                ps = psum.tile([128, 512], FP, tag="cps", name="psa", bufs=2)
                wts = wtm[m]
                for k in range(KC):
                    nc.tensor.matmul(
                        ps[:],
                        _fr(wts[:, k * 128 : (k + 1) * 128]),
                        _fr(xs[half][q2][:, k * 512 : (k + 1) * 512]),
                        start=(k == 0), stop=(k == KC - 1),
                    )
                qlo = half * 1024 + q2 * 512
                nc.vector.tensor_copy(dstT[m][:, qlo : qlo + 512], ps[:])

            def v_chunk(s_idx):
                half, sc = s_idx // 8, s_idx % 8
                ps = psum.tile([128, 512], FP, tag="cps", name="psv", bufs=2)
                j, off = sc // 4, (sc % 4) * 128
                for k in range(KC):
                    nc.tensor.matmul(
                        ps[:],
                        _fr(xs[half][j][:, k * 512 + off : k * 512 + off + 128]),
                        _fr(wv_all[:, k * LD : (k + 1) * LD]),
                        start=(k == 0), stop=(k == KC - 1),
                    )
                src = ps[:].rearrange("p (h e) -> p h e", h=LH)
                dst = vt[s_idx][:].rearrange("p (h e) -> p h e", e=E1)[:, :, 0:DH]
                nc.vector.tensor_copy(dst, src)

            def A(fn, *args):
                return lambda: fn(*args)

            # minimal head: K(m0,h0,q2=0) covers logits kc0..3, Q(m0) first
            # query slice, V(s0..s3) covers the first AV chunks
            qk_chunk(0, wkm, kt, 0, 0)
            qk_chunk(0, wqm, qt, 0, 0)
            for s in range(4):
                v_chunk(s)

            # (0,0) drip, rate 1: position p pops after logits(p+2); V_s must
            # sit at position <= s-1, K(m0,h*,q2) before logits emission of
            # its key range (kc4@slot2, kc8@slot6, kc12@slot10).
            d00 = [
                A(qk_chunk, 0, wkm, kt, 0, 1),  # keys 512:1024 (kc4+)
                A(v_chunk, 4), A(v_chunk, 5), A(v_chunk, 6),
                A(qk_chunk, 1, wkm, kt, 0, 0),  # keys 1024:1536 (kc8+)
                A(v_chunk, 7), A(v_chunk, 8), A(v_chunk, 9),
                A(qk_chunk, 1, wkm, kt, 0, 1),  # keys 1536:2048 (kc12+)
                A(v_chunk, 10), A(v_chunk, 11), A(v_chunk, 12), A(v_chunk, 13),
                A(v_chunk, 14), A(v_chunk, 15),
                A(qk_chunk, 0, wqm, qt, 0, 1),  # qt1 slice for B(0,1)
            ]
            # later head pairs: K(m) chunks drip inside (m,0) itself, just
            # ahead of the logits emission for their key range
            sched = {
                (0, 1): [A(qk_chunk, 1, wqm, qt, 0, 0), A(qk_chunk, 1, wqm, qt, 0, 1)],
                (0, 3): [A(qk_chunk, 0, wkm, kt, 1, 0), A(qk_chunk, 0, wqm, qt, 1, 0)],
                (1, 0): [A(qk_chunk, 0, wkm, kt, 1, 1), A(qk_chunk, 1, wkm, kt, 1, 0),
                         A(qk_chunk, 1, wkm, kt, 1, 1), None,
                         A(qk_chunk, 0, wqm, qt, 1, 1)],
                (1, 1): [A(qk_chunk, 1, wqm, qt, 1, 0), A(qk_chunk, 1, wqm, qt, 1, 1)],
                (1, 3): [A(qk_chunk, 0, wkm, kt, 2, 0), A(qk_chunk, 0, wqm, qt, 2, 0)],
                (2, 0): [A(qk_chunk, 0, wkm, kt, 2, 1), A(qk_chunk, 1, wkm, kt, 2, 0),
                         A(qk_chunk, 1, wkm, kt, 2, 1), None,
                         A(qk_chunk, 0, wqm, qt, 2, 1)],
                (2, 1): [A(qk_chunk, 1, wqm, qt, 2, 0), A(qk_chunk, 1, wqm, qt, 2, 1)],
                (2, 2): [A(qk_chunk, 0, wkm, kt, 3, 0), A(qk_chunk, 0, wkm, kt, 3, 1)],
                (2, 3): [A(qk_chunk, 1, wkm, kt, 3, 0), A(qk_chunk, 1, wkm, kt, 3, 1),
                         A(qk_chunk, 0, wqm, qt, 3, 0)],
                (3, 0): [A(qk_chunk, 0, wqm, qt, 3, 1), A(qk_chunk, 1, wqm, qt, 3, 0),
                         A(qk_chunk, 1, wqm, qt, 3, 1)],
            }

            wo_all = w_pool.tile([128, MC * D], BF, tag="wo", name="wo_all")
            nc.sync.dma_start(
                wo_all[:].rearrange("r (j c) -> r j c", j=MC),
                wo.rearrange("(j r) c -> r j c", r=128),
            )

            drip = deque(d00)
            tpf = emit_B(0, 0, drip, rate=1)
            for m in range(4):
                for qti in range(QT):
                    if m == 0 and qti == 0:
                        continue
                    items = list(sched.get((m, qti), []))
                    if m == 3 and qti > 0:
                        for qc in range(4 * (qti - 1), 4 * qti):
                            for n in range(2):
                                items.append(c_group(qc, n, wo_all))
                    if None in items:
                        i = items.index(None)
                        drip.extend(items[:i])
                        drip.append(tpf)
                        drip.extend(items[i + 1 :])
                    else:
                        drip.append(tpf)
                        drip.extend(items)
                    tpf = emit_B(m, qti, drip)
            # tail: last transposes + last output-projection tile
            tpf()
            for qc in range(12, 16):
                for n in range(2):
                    c_group(qc, n, wo_all)()


_nc = None


def get_nc():
    global _nc
    if _nc is None:
        _nc = build_nc()
    return _nc


def make_in_maps(x, mask, Wq, Wk, Wv, Wo):
    import ml_dtypes

    x = np.asarray(x, dtype=np.float32)
    mask = np.asarray(mask)
    Wq, Wk, Wv, Wo = (np.asarray(w, dtype=np.float32) for w in (Wq, Wk, Wv, Wo))
    in_maps = []
    for c in range(N_CORES):
        b, hg = c // HG, c % HG
        lo, hi = hg * LD, (hg + 1) * LD
        kb = np.where(mask[b], 0.0, NEG_INF).astype(np.float32)
        in_maps.append(
            {
                "xT": np.ascontiguousarray(x[b].T),
                "wq": np.ascontiguousarray(Wq[:, lo:hi]),
                "wk": np.ascontiguousarray(Wk[:, lo:hi]),
                "wv": np.ascontiguousarray(Wv[:, lo:hi]),
                "wo": np.ascontiguousarray(Wo[lo:hi, :]).astype(ml_dtypes.bfloat16),
                "kbias": np.ascontiguousarray(kb.reshape(SC, 128).T),
                "idn": np.eye(128, dtype=np.float32).astype(ml_dtypes.bfloat16),
            }
        )
    return in_maps


def kernel(x, mask, Wq, Wk, Wv, Wo):
    nc = get_nc()
    in_maps = make_in_maps(x, mask, Wq, Wk, Wv, Wo)
    res = run_bass_kernel_spmd(nc, in_maps, list(range(N_CORES)))
    outs = np.empty((B, S, D), dtype=np.float32)
    for b in range(B):
        outs[b] = res.results[2 * b]["out"] + res.results[2 * b + 1]["out"]
    return outs


# revision 3
# speedup vs baseline: 1.3774x; 1.0291x over previous
"""Multi-head attention forward on 8 Trainium2 NeuronCores — v3.

Like v2 (AV in [q,d] orientation, bf16 Q/K/attn/Wo, per-partition normalize,
PE transposes) but with pipeline-aware emission for the in-order engines:

- Within a phase (head pair m, query tile qt), logits+exp for key chunk kc+2 are
  emitted BEFORE the AV matmuls of chunk kc, so the ACT engine always has two
  exp instructions in flight when an AV matmul blocks in the PE wait queue.
- The attn transposes of phase i are deferred into phase i+1 (through the cps
  PSUM ring, not the lg ring), so they never delay the next phase's logits.
- Stage-A projection chunks and stage-C output-projection groups drip one per
  key chunk through the cps ring, placed after the logits emission point.
- PSUM: lg [128,1024]x2 (4 banks), avA/avB [128,260]x1 (2), cps [128,512]x2 (2).
"""
import sys

sys.path.insert(0, "/opt/trn_rl_repo")

import numpy as np
from collections import deque

import concourse.bass as bass
import concourse.tile as tile
from concourse import mybir
from concourse.bass_utils import run_bass_kernel_spmd
from concourse.vector_clock import ScopedClock

_wsplit_ctr = [0]


def split_multi_waits(nc):
    """Walrus accepts at most one sync wait per instruction; split extras
    into single-wait nops."""
    for f in nc.m.functions:
        for bb in f.blocks:
            out = []
            changed = False
            for inst in bb.instructions:
                si = inst.sync_info
                waits = list(si.on_wait) if si is not None and si.on_wait else []
                if len(waits) > 1:
                    updates = list(si.on_update) if si.on_update else []
                    for w in waits[1:]:
                        _wsplit_ctr[0] += 1
                        nop = mybir.InstNoOp(
                            name=f"I-wsplit-{_wsplit_ctr[0]}", ins=[], outs=[]
                        )
                        nop.engine = inst.engine
                        nop.sync_info = mybir.SyncInfo(on_wait=[w], on_update=[])
                        out.append(nop)
                    inst.sync_info = mybir.SyncInfo(on_wait=[waits[0]], on_update=updates)
                    changed = True
                out.append(inst)
            if changed:
                bb.instructions = out
    return nc


B, S, D, H, DH = 4, 2048, 1024, 16, 64
HG = 2
LD = D // HG
LH = H // HG
N_CORES = B * HG
SCALE = float(DH) ** -0.5
NEG_INF = -1e30

FP = mybir.dt.float32
FPR = mybir.dt.float32r
BF = mybir.dt.bfloat16

KC = D // 128
MC = LD // 128
SC = S // 128
QT = S // 512
Exp = mybir.ActivationFunctionType.Exp
E1 = DH + 1


def _fr(ap):
    return ap.bitcast(FPR)


class SplitDrainTileContext(tile.TileContext):
    def _drain_and_barrier(self, tick_clock, wait_clock):
        nc = self.nc
        probe = nc.sync.nop()
        wait_clock.add_sem_waits(
            probe.ins, ScopedClock({None: tick_clock.global_clock})
        )
        si = probe.ins.sync_info
        waits = list(si.on_wait) if si is not None and si.on_wait else []
        updates = list(si.on_update) if si is not None and si.on_update else []
        if len(waits) > 1:
            probe.ins.sync_info = mybir.SyncInfo(on_wait=[waits[0]], on_update=updates)
            for w in waits[1:]:
                n2 = nc.sync.nop()
                n2.ins.sync_info = mybir.SyncInfo(on_wait=[w], on_update=[])
        nc.sync.drain()
        nc.all_engine_barrier()
        popped = nc._tile_sem_poison_stack.pop()
        assert popped is self._sem_poison
        nc.clear_and_free_semaphores(list(self.sems.allocated().values()))
        nc.all_engine_barrier()


def build_nc(for_hw=True):
    nc = bass.Bass(trn_type="TRN2")
    xT = nc.dram_tensor("xT", [D, S], FP, kind="ExternalInput").ap()
    wq = nc.dram_tensor("wq", [D, LD], FP, kind="ExternalInput").ap()
    wk = nc.dram_tensor("wk", [D, LD], FP, kind="ExternalInput").ap()
    wv = nc.dram_tensor("wv", [D, LD], FP, kind="ExternalInput").ap()
    wo = nc.dram_tensor("wo", [LD, D], BF, kind="ExternalInput").ap()
    kbias = nc.dram_tensor("kbias", [128, SC], FP, kind="ExternalInput").ap()
    idn = nc.dram_tensor("idn", [128, 128], BF, kind="ExternalInput").ap()
    out = nc.dram_tensor("out", [S, D], FP, kind="ExternalOutput").ap()

    with SplitDrainTileContext(nc) as tc:
        _body(tc, xT, wq, wk, wv, wo, kbias, idn, out)
    if for_hw:
        split_multi_waits(nc)
    return nc


def _body(tc, xT, wq, wk, wv, wo, kbias, idn, out):
    nc = tc.nc
    with (
        tc.tile_pool(name="pers", bufs=1) as pers,
        tc.tile_pool(name="pt", bufs=4) as pt_pool,
        tc.tile_pool(name="a2", bufs=8) as a2_pool,
        tc.tile_pool(name="rs", bufs=4) as rs_pool,
        tc.tile_pool(name="ot", bufs=3) as ot_pool,
        tc.tile_pool(name="psmm", bufs=1, space="PSUM") as psum,
    ):
        qt = [pers.tile([128, S], BF, tag=f"qt{m}", name=f"qt{m}") for m in range(MC)]
        kt = [pers.tile([128, S], BF, tag=f"kt{m}", name=f"kt{m}") for m in range(MC)]
        vt = [pers.tile([128, LH * E1], BF, tag=f"v{s}", name=f"v{s}") for s in range(SC)]
        attT = [pers.tile([128, S], BF, tag=f"at{m}", name=f"at{m}") for m in range(MC)]
        biasT = pers.tile([128, SC], FP, tag="biasT")
        idn_sb = pers.tile([128, 128], BF, tag="idn")

        for s in range(SC):
            dst = vt[s][:].rearrange("p (h e) -> p h e", e=E1)[:, :, DH : DH + 1]
            nc.vector.memset(dst, 1.0)

        # ---------- stage B phase emitter ----------
        def emit_B(m, qti, drip, rate=2):
            """One attention phase. Emits logits/exp two key-chunks ahead of
            the AV matmuls; pops drip closures after the logits point of
            every `rate`-th key chunk."""
            hA, hB = 2 * m, 2 * m + 1
            qs = slice(qti * 512, (qti + 1) * 512)
            avA = psum.tile([128, 4 * E1], FP, tag="avA", name="avA", bufs=1)
            avB = psum.tile([128, 4 * E1], FP, tag="avB", name="avB", bufs=1)
            pts = {}

            def logits(kc):
                ks = slice(kc * 128, (kc + 1) * 128)
                lg = psum.tile([128, 1024], FP, tag="lg", name="lg", bufs=2)
                nc.tensor.matmul(
                    lg[:, 0:512], kt[m][0:64, ks], qt[m][0:64, qs],
                    start=True, stop=True,
                )
                nc.tensor.matmul(
                    lg[:, 512:1024], kt[m][64:128, ks], qt[m][64:128, qs],
                    start=True, stop=True,
                )
                pt = pt_pool.tile([128, 1024], BF, tag="pt", name="pt")
                nc.scalar.activation(
                    pt[:], lg[:], Exp, bias=biasT[:, kc : kc + 1], scale=SCALE
                )
                pts[kc] = pt

            logits(0)
            logits(1)
            for kc in range(SC):
                pt = pts.pop(kc)
                for qc in range(4):
                    # start=True only on the first slice: the PSUM zero-region
                    # "pending zero" marking spans the whole bank, so later
                    # start=True calls would wipe sibling slices' first chunk
                    nc.tensor.matmul(
                        avA[:, qc * E1 : (qc + 1) * E1],
                        pt[:, qc * 128 : (qc + 1) * 128],
                        vt[kc][:, hA * E1 : (hA + 1) * E1],
                        start=(kc == 0 and qc == 0), stop=(kc == SC - 1),
                        skip_group_check=True,
                    )
                    nc.tensor.matmul(
                        avB[:, qc * E1 : (qc + 1) * E1],
                        pt[:, 512 + qc * 128 : 512 + (qc + 1) * 128],
                        vt[kc][:, hB * E1 : (hB + 1) * E1],
                        start=(kc == 0 and qc == 0), stop=(kc == SC - 1),
                        skip_group_check=True,
                    )
                if kc + 2 < SC:
                    logits(kc + 2)
                if drip and kc % rate == rate - 1:
                    drip.popleft()()
            while drip:
                drip.popleft()()
            # normalize into a2 staging tiles (bf16); transposes are deferred
            rsA = rs_pool.tile([128, 4], FP, tag="rs", name="rsA")
            rsB = rs_pool.tile([128, 4], FP, tag="rs", name="rsB")
            avAr = avA[:].rearrange("p (q e) -> p q e", e=E1)
            avBr = avB[:].rearrange("p (q e) -> p q e", e=E1)
            a2s = [a2_pool.tile([128, 128], BF, tag="a2", name="a2") for _ in range(4)]
            nc.vector.reciprocal(rsA[:], avAr[:, :, DH : DH + 1])
            for qc in range(4):
                nc.vector.tensor_scalar(
                    a2s[qc][:, 0:64], avA[:, qc * E1 : qc * E1 + DH],
                    rsA[:, qc : qc + 1], None, mybir.AluOpType.mult,
                )
            nc.vector.reciprocal(rsB[:], avBr[:, :, DH : DH + 1])
            for qc in range(4):
                nc.vector.tensor_scalar(
                    a2s[qc][:, 64:128], avB[:, qc * E1 : qc * E1 + DH],
                    rsB[:, qc : qc + 1], None, mybir.AluOpType.mult,
                )

            def tp_flush(m=m, qs=qs, a2s=a2s):
                tp = psum.tile([128, 512], BF, tag="cps", name="tp", bufs=2)
                for qc in range(4):
                    nc.tensor.matmul(
                        tp[:, qc * 128 : (qc + 1) * 128], a2s[qc][:], idn_sb[:],
                        is_transpose=True, skip_group_check=True,
                    )
                nc.vector.tensor_copy(attT[m][:, qs], tp[:])

            return tp_flush

        def c_group(qc, n, wo_all, tag="cps", eng=None):
            def emit():
                cps = psum.tile([128, 512], FP, tag=tag, name="cps", bufs=2)
                for j in range(MC):
                    nc.tensor.matmul(
                        cps[:],
                        attT[j][:, qc * 128 : (qc + 1) * 128],
                        wo_all[:, j * D + n * 512 : j * D + (n + 1) * 512],
                        start=(j == 0), stop=(j == MC - 1),
                    )
                ot = ot_pool.tile([128, 512], FP, tag="ot", name="ot")
                if eng == "act":
                    nc.scalar.copy(ot[:], cps[:])
                else:
                    nc.vector.tensor_copy(ot[:], cps[:])
                nc.sync.dma_start(
                    out[qc * 128 : (qc + 1) * 128, n * 512 : (n + 1) * 512],
                    ot[:],
                )

            return emit

        # ---------- stage A ----------
        with (
            tc.tile_pool(name="xt", bufs=1) as xt_pool,
            tc.tile_pool(name="w", bufs=1) as w_pool,
        ):
            # batched tiles: one DMA each (HWDGE gen is ~650ns per dma_start)
            wkm = [w_pool.tile([128, KC * 128], FP, tag=f"wkm{m}", name=f"wkm{m}") for m in range(MC)]
            wqm = [w_pool.tile([128, KC * 128], FP, tag=f"wqm{m}", name=f"wqm{m}") for m in range(MC)]
            wv_all = w_pool.tile([128, KC * LD], FP, tag="wv", name="wv_all")
            xs = [
                [xt_pool.tile([128, KC * 512], FP, tag=f"x{h}_{j}", name=f"x{h}_{j}") for j in range(2)]
                for h in range(2)
            ]
            wkR = wk.rearrange("(k r) c -> r k c", r=128)
            wqR = wq.rearrange("(k r) c -> r k c", r=128)
            wvR = wv.rearrange("(k r) c -> r k c", r=128)
            xR = xT.rearrange("(k r) q -> r k q", r=128)

            def xv(t):  # [128, (k c)] view
                return t[:].rearrange("r (k c) -> r k c", k=KC)

            # priority order: inputs of the head chunks first, split by
            # contraction half so the first accumulation chain starts early
            for ks_ in (slice(0, 4), slice(4, 8)):
                nc.sync.dma_start(_fr(xv(wkm[0])[:, ks_]), _fr(wkR[:, ks_, 0:128]))
                nc.sync.dma_start(_fr(xv(xs[0][0])[:, ks_]), _fr(xR[:, ks_, 0:512]))
                nc.sync.dma_start(_fr(xv(wqm[0])[:, ks_]), _fr(wqR[:, ks_, 0:128]))
            nc.sync.dma_start(_fr(xv(wv_all)), _fr(wvR[:, :, :]))
            nc.sync.dma_start(biasT[:], kbias[:])
            nc.sync.dma_start(_fr(xv(xs[0][1])), _fr(xR[:, :, 512:1024]))
            for h, j in ((1, 0), (1, 1)):
                lo = h * 1024 + j * 512
                nc.sync.dma_start(_fr(xv(xs[h][j])), _fr(xR[:, :, lo : lo + 512]))
            for m in range(1, MC):
                nc.sync.dma_start(_fr(xv(wkm[m])), _fr(wkR[:, :, m * 128 : (m + 1) * 128]))
                nc.sync.dma_start(_fr(xv(wqm[m])), _fr(wqR[:, :, m * 128 : (m + 1) * 128]))
            nc.sync.dma_start(idn_sb[:], idn[:])

            def qk_chunk(half, wtm, dstT, m, q2):
                ps = psum.tile([128, 512], FP, tag="cps", name="psa", bufs=2)
                wts = wtm[m]
                for k in range(KC):
                    nc.tensor.matmul(
                        ps[:],
                        _fr(wts[:, k * 128 : (k + 1) * 128]),
                        _fr(xs[half][q2][:, k * 512 : (k + 1) * 512]),
                        start=(k == 0), stop=(k == KC - 1),
                    )
                qlo = half * 1024 + q2 * 512
                nc.vector.tensor_copy(dstT[m][:, qlo : qlo + 512], ps[:])

            def v_chunk(s_idx):
                half, sc = s_idx // 8, s_idx % 8
                ps = psum.tile([128, 512], FP, tag="cps", name="psv", bufs=2)
                j, off = sc // 4, (sc % 4) * 128
                for k in range(KC):
                    nc.tensor.matmul(
                        ps[:],
                        _fr(xs[half][j][:, k * 512 + off : k * 512 + off + 128]),
                        _fr(wv_all[:, k * LD : (k + 1) * LD]),
                        start=(k == 0), stop=(k == KC - 1),
                    )
                src = ps[:].rearrange("p (h e) -> p h e", h=LH)
                dst = vt[s_idx][:].rearrange("p (h e) -> p h e", e=E1)[:, :, 0:DH]
                nc.vector.tensor_copy(dst, src)

            def A(fn, *args):
                return lambda: fn(*args)

            # minimal head: K(m0,h0,q2=0) covers logits kc0..3, Q(m0) first
            # query slice, V(s0..s3) covers the first AV chunks
            qk_chunk(0, wkm, kt, 0, 0)
            qk_chunk(0, wqm, qt, 0, 0)
            for s in range(4):
                v_chunk(s)

            wo_all = w_pool.tile([128, MC * D], BF, tag="wo", name="wo_all")
            nc.sync.dma_start(
                wo_all[:].rearrange("r (j c) -> r j c", j=MC),
                wo.rearrange("(j r) c -> r j c", r=128),
            )

            # (0,0) drip, rate 1: position p pops after logits(p+2); V_s must
            # sit at position <= s-1, K(m0,h*,q2) before logits emission of
            # its key range (kc4@slot2, kc8@slot6, kc12@slot10).
            d00 = [
                A(qk_chunk, 0, wkm, kt, 0, 1),  # keys 512:1024 (kc4+)
                A(v_chunk, 4), A(v_chunk, 5), A(v_chunk, 6),
                A(qk_chunk, 1, wkm, kt, 0, 0),  # keys 1024:1536 (kc8+)
                A(v_chunk, 7), A(v_chunk, 8), A(v_chunk, 9),
                A(qk_chunk, 1, wkm, kt, 0, 1),  # keys 1536:2048 (kc12+)
                A(v_chunk, 10), A(v_chunk, 11), A(v_chunk, 12), A(v_chunk, 13),
                A(v_chunk, 14), A(v_chunk, 15),
                A(qk_chunk, 0, wqm, qt, 0, 1),  # qt1 slice for (0,1)
            ]

            def C(qti, lo, hi, tag="cps"):  # out-proj groups [lo,hi) of tile qti
                return [c_group(qc, n, wo_all, tag)
                        for g in range(lo, hi)
                        for qc, n in [(4 * qti + g // 2, g % 2)]]

            K_ = lambda m, h, q2: A(qk_chunk, h, wkm, kt, m, q2)
            Q_ = lambda m, h, q2: A(qk_chunk, h, wqm, qt, m, q2)
            # anti-diagonal phase rotation: stage-A chunks spread so nearly
            # every phase stays ACT(exp)-paced; None marks where the previous
            # phase's deferred transpose-flush goes.
            rotation = [
                ((0, 1), [None, K_(1, 0, 0), Q_(1, 0, 0), Q_(0, 1, 0)]),
                ((1, 0), [K_(1, 0, 1), K_(1, 1, 0), K_(1, 1, 1), None, Q_(0, 1, 1)]),
                ((0, 2), [None, Q_(1, 0, 1), K_(2, 0, 0)]),
                ((1, 1), [None, Q_(2, 0, 0), Q_(1, 1, 0)]),
                ((2, 0), [K_(2, 0, 1), K_(2, 1, 0), K_(2, 1, 1), None]),
                ((0, 3), [None, Q_(2, 0, 1)]),
                ((1, 2), [None, K_(3, 0, 0), Q_(3, 0, 0)]),
                ((2, 1), [None, Q_(1, 1, 1)]),
                ((3, 0), [K_(3, 0, 1), K_(3, 1, 0), K_(3, 1, 1), None, Q_(2, 1, 0)]),
                ((1, 3), [None, Q_(3, 0, 1)] + C(0, 0, 2)),
                ((2, 2), [None, Q_(2, 1, 1)] + C(0, 2, 6)),
                ((3, 1), [None, Q_(3, 1, 0)] + C(0, 6, 8)),
                ((3, 2), [None, Q_(3, 1, 1)] + C(1, 0, 5)),
                ((2, 3), [None] + C(1, 5, 8)),
                ((3, 3), [None] + C(2, 0, 8)),
            ]

            drip = deque(d00)
            tpf = emit_B(0, 0, drip, rate=1)
            for (m, qti), items in rotation:
                i = items.index(None)
                drip.extend(items[:i])
                drip.append(tpf)
                drip.extend(items[i + 1 :])
                tpf = emit_B(m, qti, drip)
            # tail: last transpose-flush + last query tile's output projection,
            # alternating PSUM rings (lg ring is free after the last exp)
            tpf()
            for i in range(8):
                c_group(12 + i // 2, i % 2, wo_all, "lg" if i % 2 else "cps",
                        eng="act" if i % 2 else None)()


_nc = None


def get_nc():
    global _nc
    if _nc is None:
        _nc = build_nc()
    return _nc


def make_in_maps(x, mask, Wq, Wk, Wv, Wo):
    import ml_dtypes

    x = np.asarray(x, dtype=np.float32)
    mask = np.asarray(mask)
    Wq, Wk, Wv, Wo = (np.asarray(w, dtype=np.float32) for w in (Wq, Wk, Wv, Wo))
    in_maps = []
    for c in range(N_CORES):
        b, hg = c // HG, c % HG
        lo, hi = hg * LD, (hg + 1) * LD
        kb = np.where(mask[b], 0.0, NEG_INF).astype(np.float32)
        in_maps.append(
            {
                "xT": np.ascontiguousarray(x[b].T),
                "wq": np.ascontiguousarray(Wq[:, lo:hi]),
                "wk": np.ascontiguousarray(Wk[:, lo:hi]),
                "wv": np.ascontiguousarray(Wv[:, lo:hi]),
                "wo": np.ascontiguousarray(Wo[lo:hi, :]).astype(ml_dtypes.bfloat16),
                "kbias": np.ascontiguousarray(kb.reshape(SC, 128).T),
                "idn": np.eye(128, dtype=np.float32).astype(ml_dtypes.bfloat16),
            }
        )
    return in_maps


def kernel(x, mask, Wq, Wk, Wv, Wo):
    nc = get_nc()
    in_maps = make_in_maps(x, mask, Wq, Wk, Wv, Wo)
    res = run_bass_kernel_spmd(nc, in_maps, list(range(N_CORES)))
    outs = np.empty((B, S, D), dtype=np.float32)
    for b in range(B):
        outs[b] = res.results[2 * b]["out"] + res.results[2 * b + 1]["out"]
    return outs


# revision 4
# speedup vs baseline: 1.3786x; 1.0009x over previous
"""Multi-head attention forward on 8 Trainium2 NeuronCores.

Problem: x[4,2048,1024], 16 heads (d=64), fp32. out = softmax(QK^T/sqrt(d) + mask) V @ Wo.

Sharding: core = (batch b in 0..3) x (head-group hg in 0..1); each core does one batch
element and 8 heads (a 512-wide model-dim slice), emitting a partial [2048,1024] that
the host sums per batch element.

Design (cost-model driven):
- AV matmul in [q, d] orientation: out free dim = 65 (64 d + a ones column that yields
  the softmax row sums for free), with q on the 128 output partitions -> half the PE
  time of the transposed orientation. Small bf16 PE transposes flip the normalized
  attention back to [d, q] for the output projection.
- Softmax normalization via per-partition tensor_scalar with the reciprocal rowsum.
- Q/K/attn/Wo in bf16 (same PE rate as fp32r, half the SBUF, ~0.3% noise);
  projections and logits accumulate in fp32.
- PSUM (8 banks): lg [128,1024]x2 ping-pong for logits/exp (4), avA/avB accumulation
  (2), cps ring (2) shared by stage-A projection chunks, deferred transposes and
  stage-C groups. Only the first slice of an av accumulation issues start=True: the
  PSUM "pending zero" marking spans the whole 2KB zero region, so a later start=True
  would wipe sibling slices' first contribution.
- Pipeline-aware emission for the in-order engines: per phase (head pair, query tile),
  logits+exp for key chunk kc+2 are emitted before the AV matmuls of chunk kc (the ACT
  engine always has two exp in flight when an AV blocks in the PE wait queue);
  transposes are deferred into the next phase; stage-A chunks and stage-C groups drip
  through the phases in an anti-diagonal rotation so nearly every phase stays
  exp-paced, with batched priority-ordered DMAs feeding the head of the pipeline.
"""
import sys

sys.path.insert(0, "/opt/trn_rl_repo")

import numpy as np
from collections import deque

import concourse.bass as bass
import concourse.tile as tile
from concourse import mybir
from concourse.bass_utils import run_bass_kernel_spmd
from concourse.vector_clock import ScopedClock

_wsplit_ctr = [0]


def split_multi_waits(nc):
    """Walrus accepts at most one sync wait per instruction; split extras
    into single-wait nops."""
    for f in nc.m.functions:
        for bb in f.blocks:
            out = []
            changed = False
            for inst in bb.instructions:
                si = inst.sync_info
                waits = list(si.on_wait) if si is not None and si.on_wait else []
                if len(waits) > 1:
                    updates = list(si.on_update) if si.on_update else []
                    for w in waits[1:]:
                        _wsplit_ctr[0] += 1
                        nop = mybir.InstNoOp(
                            name=f"I-wsplit-{_wsplit_ctr[0]}", ins=[], outs=[]
                        )
                        nop.engine = inst.engine
                        nop.sync_info = mybir.SyncInfo(on_wait=[w], on_update=[])
                        out.append(nop)
                    inst.sync_info = mybir.SyncInfo(on_wait=[waits[0]], on_update=updates)
                    changed = True
                out.append(inst)
            if changed:
                bb.instructions = out
    return nc


B, S, D, H, DH = 4, 2048, 1024, 16, 64
HG = 2
LD = D // HG
LH = H // HG
N_CORES = B * HG
SCALE = float(DH) ** -0.5
NEG_INF = -1e30

FP = mybir.dt.float32
FPR = mybir.dt.float32r
BF = mybir.dt.bfloat16

KC = D // 128
MC = LD // 128
SC = S // 128
QT = S // 512
Exp = mybir.ActivationFunctionType.Exp
E1 = DH + 1


def _fr(ap):
    return ap.bitcast(FPR)


class SplitDrainTileContext(tile.TileContext):
    def _drain_and_barrier(self, tick_clock, wait_clock):
        nc = self.nc
        probe = nc.sync.nop()
        wait_clock.add_sem_waits(
            probe.ins, ScopedClock({None: tick_clock.global_clock})
        )
        si = probe.ins.sync_info
        waits = list(si.on_wait) if si is not None and si.on_wait else []
        updates = list(si.on_update) if si is not None and si.on_update else []
        if len(waits) > 1:
            probe.ins.sync_info = mybir.SyncInfo(on_wait=[waits[0]], on_update=updates)
            for w in waits[1:]:
                n2 = nc.sync.nop()
                n2.ins.sync_info = mybir.SyncInfo(on_wait=[w], on_update=[])
        nc.sync.drain()
        nc.all_engine_barrier()
        popped = nc._tile_sem_poison_stack.pop()
        assert popped is self._sem_poison
        nc.clear_and_free_semaphores(list(self.sems.allocated().values()))
        nc.all_engine_barrier()


def build_nc(for_hw=True):
    nc = bass.Bass(trn_type="TRN2")
    xT = nc.dram_tensor("xT", [D, S], FP, kind="ExternalInput").ap()
    wq = nc.dram_tensor("wq", [D, LD], FP, kind="ExternalInput").ap()
    wk = nc.dram_tensor("wk", [D, LD], FP, kind="ExternalInput").ap()
    wv = nc.dram_tensor("wv", [D, LD], FP, kind="ExternalInput").ap()
    wo = nc.dram_tensor("wo", [LD, D], BF, kind="ExternalInput").ap()
    kbias = nc.dram_tensor("kbias", [128, SC], FP, kind="ExternalInput").ap()
    idn = nc.dram_tensor("idn", [128, 128], BF, kind="ExternalInput").ap()
    out = nc.dram_tensor("out", [S, D], FP, kind="ExternalOutput").ap()

    with SplitDrainTileContext(nc) as tc:
        _body(tc, xT, wq, wk, wv, wo, kbias, idn, out)
    if for_hw:
        split_multi_waits(nc)
    return nc


def _body(tc, xT, wq, wk, wv, wo, kbias, idn, out):
    nc = tc.nc
    with (
        tc.tile_pool(name="pers", bufs=1) as pers,
        tc.tile_pool(name="pt", bufs=4) as pt_pool,
        tc.tile_pool(name="a2", bufs=8) as a2_pool,
        tc.tile_pool(name="rs", bufs=4) as rs_pool,
        tc.tile_pool(name="ot", bufs=3) as ot_pool,
        tc.tile_pool(name="psmm", bufs=1, space="PSUM") as psum,
    ):
        qt = [pers.tile([128, S], BF, tag=f"qt{m}", name=f"qt{m}") for m in range(MC)]
        kt = [pers.tile([128, S], BF, tag=f"kt{m}", name=f"kt{m}") for m in range(MC)]
        vt = [pers.tile([128, LH * E1], BF, tag=f"v{s}", name=f"v{s}") for s in range(SC)]
        attT = [pers.tile([128, S], BF, tag=f"at{m}", name=f"at{m}") for m in range(MC)]
        biasT = pers.tile([128, SC], FP, tag="biasT")
        idn_sb = pers.tile([128, 128], BF, tag="idn")

        for s in range(SC):
            dst = vt[s][:].rearrange("p (h e) -> p h e", e=E1)[:, :, DH : DH + 1]
            nc.vector.memset(dst, 1.0)

        # ---------- stage B phase emitter ----------
        def emit_B(m, qti, drip, rate=2):
            """One attention phase. Emits logits/exp two key-chunks ahead of
            the AV matmuls; pops drip closures after the logits point of
            every `rate`-th key chunk."""
            hA, hB = 2 * m, 2 * m + 1
            qs = slice(qti * 512, (qti + 1) * 512)
            avA = psum.tile([128, 4 * E1], FP, tag="avA", name="avA", bufs=1)
            avB = psum.tile([128, 4 * E1], FP, tag="avB", name="avB", bufs=1)
            pts = {}

            def logits(kc):
                ks = slice(kc * 128, (kc + 1) * 128)
                lg = psum.tile([128, 1024], FP, tag="lg", name="lg", bufs=2)
                nc.tensor.matmul(
                    lg[:, 0:512], kt[m][0:64, ks], qt[m][0:64, qs],
                    start=True, stop=True,
                )
                nc.tensor.matmul(
                    lg[:, 512:1024], kt[m][64:128, ks], qt[m][64:128, qs],
                    start=True, stop=True,
                )
                pt = pt_pool.tile([128, 1024], BF, tag="pt", name="pt")
                nc.scalar.activation(
                    pt[:], lg[:], Exp, bias=biasT[:, kc : kc + 1], scale=SCALE
                )
                pts[kc] = pt

            logits(0)
            logits(1)
            for kc in range(SC):
                pt = pts.pop(kc)
                for qc in range(4):
                    # start=True only on the first slice: the PSUM zero-region
                    # "pending zero" marking spans the whole bank, so later
                    # start=True calls would wipe sibling slices' first chunk
                    nc.tensor.matmul(
                        avA[:, qc * E1 : (qc + 1) * E1],
                        pt[:, qc * 128 : (qc + 1) * 128],
                        vt[kc][:, hA * E1 : (hA + 1) * E1],
                        start=(kc == 0 and qc == 0), stop=(kc == SC - 1),
                        skip_group_check=True,
                    )
                    nc.tensor.matmul(
                        avB[:, qc * E1 : (qc + 1) * E1],
                        pt[:, 512 + qc * 128 : 512 + (qc + 1) * 128],
                        vt[kc][:, hB * E1 : (hB + 1) * E1],
                        start=(kc == 0 and qc == 0), stop=(kc == SC - 1),
                        skip_group_check=True,
                    )
                if kc + 2 < SC:
                    logits(kc + 2)
                if drip and kc % rate == rate - 1 and (rate == 1 or kc < SC - 2):
                    drip.popleft()()
            # normalize into a2 staging tiles (bf16); transposes are deferred
            rsA = rs_pool.tile([128, 4], FP, tag="rs", name="rsA")
            rsB = rs_pool.tile([128, 4], FP, tag="rs", name="rsB")
            avAr = avA[:].rearrange("p (q e) -> p q e", e=E1)
            avBr = avB[:].rearrange("p (q e) -> p q e", e=E1)
            a2s = [a2_pool.tile([128, 128], BF, tag="a2", name="a2") for _ in range(4)]
            nc.vector.reciprocal(rsA[:], avAr[:, :, DH : DH + 1])
            for qc in range(4):
                nc.vector.tensor_scalar(
                    a2s[qc][:, 0:64], avA[:, qc * E1 : qc * E1 + DH],
                    rsA[:, qc : qc + 1], None, mybir.AluOpType.mult,
                )
            nc.vector.reciprocal(rsB[:], avBr[:, :, DH : DH + 1])
            for qc in range(4):
                nc.vector.tensor_scalar(
                    a2s[qc][:, 64:128], avB[:, qc * E1 : qc * E1 + DH],
                    rsB[:, qc : qc + 1], None, mybir.AluOpType.mult,
                )

            def tp_flush(m=m, qs=qs, a2s=a2s):
                tp = psum.tile([128, 512], BF, tag="cps", name="tp", bufs=2)
                for qc in range(4):
                    nc.tensor.matmul(
                        tp[:, qc * 128 : (qc + 1) * 128], a2s[qc][:], idn_sb[:],
                        is_transpose=True, skip_group_check=True,
                    )
                nc.vector.tensor_copy(attT[m][:, qs], tp[:])

            # leftover drips emit after the normalize so their copies never
            # delay the av-ring release at the phase boundary
            while drip:
                drip.popleft()()
            return tp_flush

        def c_group(qc, n, wo_all, tag="cps", eng=None):
            def emit():
                cps = psum.tile([128, 512], FP, tag=tag, name="cps", bufs=2)
                for j in range(MC):
                    nc.tensor.matmul(
                        cps[:],
                        attT[j][:, qc * 128 : (qc + 1) * 128],
                        wo_all[:, j * D + n * 512 : j * D + (n + 1) * 512],
                        start=(j == 0), stop=(j == MC - 1),
                    )
                ot = ot_pool.tile([128, 512], FP, tag="ot", name="ot")
                if eng == "act":
                    nc.scalar.copy(ot[:], cps[:])
                else:
                    nc.vector.tensor_copy(ot[:], cps[:])
                nc.sync.dma_start(
                    out[qc * 128 : (qc + 1) * 128, n * 512 : (n + 1) * 512],
                    ot[:],
                )

            return emit

        # ---------- stage A ----------
        with (
            tc.tile_pool(name="xt", bufs=1) as xt_pool,
            tc.tile_pool(name="w", bufs=1) as w_pool,
        ):
            # batched tiles: one DMA each (HWDGE gen is ~650ns per dma_start)
            wkm = [w_pool.tile([128, KC * 128], FP, tag=f"wkm{m}", name=f"wkm{m}") for m in range(MC)]
            wqm = [w_pool.tile([128, KC * 128], FP, tag=f"wqm{m}", name=f"wqm{m}") for m in range(MC)]
            wv_all = w_pool.tile([128, KC * LD], FP, tag="wv", name="wv_all")
            xs = [
                [xt_pool.tile([128, KC * 512], FP, tag=f"x{h}_{j}", name=f"x{h}_{j}") for j in range(2)]
                for h in range(2)
            ]
            wkR = wk.rearrange("(k r) c -> r k c", r=128)
            wqR = wq.rearrange("(k r) c -> r k c", r=128)
            wvR = wv.rearrange("(k r) c -> r k c", r=128)
            xR = xT.rearrange("(k r) q -> r k q", r=128)

            def xv(t):  # [128, (k c)] view
                return t[:].rearrange("r (k c) -> r k c", k=KC)

            # priority order: inputs of the head chunks first, split by
            # contraction half so the first accumulation chain starts early
            for ks_ in (slice(0, 4), slice(4, 8)):
                nc.sync.dma_start(_fr(xv(wkm[0])[:, ks_]), _fr(wkR[:, ks_, 0:128]))
                nc.sync.dma_start(_fr(xv(xs[0][0])[:, ks_]), _fr(xR[:, ks_, 0:512]))
                nc.sync.dma_start(_fr(xv(wqm[0])[:, ks_]), _fr(wqR[:, ks_, 0:128]))
            nc.sync.dma_start(_fr(xv(wv_all)), _fr(wvR[:, :, :]))
            nc.sync.dma_start(biasT[:], kbias[:])
            nc.sync.dma_start(_fr(xv(xs[0][1])), _fr(xR[:, :, 512:1024]))
            for h, j in ((1, 0), (1, 1)):
                lo = h * 1024 + j * 512
                nc.sync.dma_start(_fr(xv(xs[h][j])), _fr(xR[:, :, lo : lo + 512]))
            for m in range(1, MC):
                nc.sync.dma_start(_fr(xv(wkm[m])), _fr(wkR[:, :, m * 128 : (m + 1) * 128]))
                nc.sync.dma_start(_fr(xv(wqm[m])), _fr(wqR[:, :, m * 128 : (m + 1) * 128]))
            nc.sync.dma_start(idn_sb[:], idn[:])

            def qk_chunk(half, wtm, dstT, m, q2):
                ps = psum.tile([128, 512], FP, tag="cps", name="psa", bufs=2)
                wts = wtm[m]
                for k in range(KC):
                    nc.tensor.matmul(
                        ps[:],
                        _fr(wts[:, k * 128 : (k + 1) * 128]),
                        _fr(xs[half][q2][:, k * 512 : (k + 1) * 512]),
                        start=(k == 0), stop=(k == KC - 1),
                    )
                qlo = half * 1024 + q2 * 512
                nc.vector.tensor_copy(dstT[m][:, qlo : qlo + 512], ps[:])

            def v_chunk(s_idx):
                half, sc = s_idx // 8, s_idx % 8
                ps = psum.tile([128, 512], FP, tag="cps", name="psv", bufs=2)
                j, off = sc // 4, (sc % 4) * 128
                for k in range(KC):
                    nc.tensor.matmul(
                        ps[:],
                        _fr(xs[half][j][:, k * 512 + off : k * 512 + off + 128]),
                        _fr(wv_all[:, k * LD : (k + 1) * LD]),
                        start=(k == 0), stop=(k == KC - 1),
                    )
                src = ps[:].rearrange("p (h e) -> p h e", h=LH)
                dst = vt[s_idx][:].rearrange("p (h e) -> p h e", e=E1)[:, :, 0:DH]
                nc.vector.tensor_copy(dst, src)

            def A(fn, *args):
                return lambda: fn(*args)

            # minimal head: K(m0,h0,q2=0) covers logits kc0..3, Q(m0) first
            # query slice, V(s0..s3) covers the first AV chunks
            qk_chunk(0, wkm, kt, 0, 0)
            qk_chunk(0, wqm, qt, 0, 0)
            for s in range(4):
                v_chunk(s)

            wo_all = w_pool.tile([128, MC * D], BF, tag="wo", name="wo_all")
            nc.sync.dma_start(
                wo_all[:].rearrange("r (j c) -> r j c", j=MC),
                wo.rearrange("(j r) c -> r j c", r=128),
            )

            # (0,0) drip, rate 1: position p pops after logits(p+2); V_s must
            # sit at position <= s-1, K(m0,h*,q2) before logits emission of
            # its key range (kc4@slot2, kc8@slot6, kc12@slot10).
            d00 = [
                A(qk_chunk, 0, wkm, kt, 0, 1),  # keys 512:1024 (kc4+)
                A(v_chunk, 4), A(v_chunk, 5), A(v_chunk, 6),
                A(qk_chunk, 1, wkm, kt, 0, 0),  # keys 1024:1536 (kc8+)
                A(v_chunk, 7), A(v_chunk, 8), A(v_chunk, 9),
                A(qk_chunk, 1, wkm, kt, 0, 1),  # keys 1536:2048 (kc12+)
                A(v_chunk, 10), A(v_chunk, 11), A(v_chunk, 12), A(v_chunk, 13),
                A(v_chunk, 14), A(v_chunk, 15),
                A(qk_chunk, 0, wqm, qt, 0, 1),  # qt1 slice for (0,1)
            ]

            def C(qti, lo, hi, tag="cps"):  # out-proj groups [lo,hi) of tile qti
                return [c_group(qc, n, wo_all, tag)
                        for g in range(lo, hi)
                        for qc, n in [(4 * qti + g // 2, g % 2)]]

            K_ = lambda m, h, q2: A(qk_chunk, h, wkm, kt, m, q2)
            Q_ = lambda m, h, q2: A(qk_chunk, h, wqm, qt, m, q2)
            # anti-diagonal phase rotation: stage-A chunks spread so nearly
            # every phase stays ACT(exp)-paced; None marks where the previous
            # phase's deferred transpose-flush goes.
            rotation = [
                ((0, 1), [None, K_(1, 0, 0), Q_(1, 0, 0), Q_(0, 1, 0)]),
                ((1, 0), [K_(1, 0, 1), K_(1, 1, 0), K_(1, 1, 1), None, Q_(0, 1, 1)]),
                ((0, 2), [None, Q_(1, 0, 1), K_(2, 0, 0)]),
                ((1, 1), [None, Q_(2, 0, 0), Q_(1, 1, 0)]),
                ((2, 0), [K_(2, 0, 1), K_(2, 1, 0), K_(2, 1, 1), None]),
                ((0, 3), [None, Q_(2, 0, 1)]),
                ((1, 2), [None, K_(3, 0, 0), Q_(3, 0, 0)]),
                ((2, 1), [None, Q_(1, 1, 1)]),
                ((3, 0), [K_(3, 0, 1), K_(3, 1, 0), K_(3, 1, 1), None, Q_(2, 1, 0)]),
                ((1, 3), [None, Q_(3, 0, 1)] + C(0, 0, 2)),
                ((2, 2), [None, Q_(2, 1, 1)] + C(0, 2, 6)),
                ((3, 1), [None, Q_(3, 1, 0)] + C(0, 6, 8)),
                ((3, 2), [None, Q_(3, 1, 1)] + C(1, 0, 5)),
                ((2, 3), [None] + C(1, 5, 8)),
                ((3, 3), [None] + C(2, 0, 8)),
            ]

            drip = deque(d00)
            tpf = emit_B(0, 0, drip, rate=1)
            for (m, qti), items in rotation:
                i = items.index(None)
                drip.extend(items[:i])
                drip.append(tpf)
                drip.extend(items[i + 1 :])
                tpf = emit_B(m, qti, drip)
            # tail: last transpose-flush + last query tile's output projection,
            # alternating PSUM rings (lg ring is free after the last exp)
            tpf()
            for i in range(8):
                c_group(12 + i // 2, i % 2, wo_all, "lg" if i % 2 else "cps",
                        eng="act" if i % 2 else None)()


_nc = None


def get_nc():
    global _nc
    if _nc is None:
        _nc = build_nc()
    return _nc


def make_in_maps(x, mask, Wq, Wk, Wv, Wo):
    import ml_dtypes

    x = np.asarray(x, dtype=np.float32)
    mask = np.asarray(mask)
    Wq, Wk, Wv, Wo = (np.asarray(w, dtype=np.float32) for w in (Wq, Wk, Wv, Wo))
    in_maps = []
    for c in range(N_CORES):
        b, hg = c // HG, c % HG
        lo, hi = hg * LD, (hg + 1) * LD
        kb = np.where(mask[b], 0.0, NEG_INF).astype(np.float32)
        in_maps.append(
            {
                "xT": np.ascontiguousarray(x[b].T),
                "wq": np.ascontiguousarray(Wq[:, lo:hi]),
                "wk": np.ascontiguousarray(Wk[:, lo:hi]),
                "wv": np.ascontiguousarray(Wv[:, lo:hi]),
                "wo": np.ascontiguousarray(Wo[lo:hi, :]).astype(ml_dtypes.bfloat16),
                "kbias": np.ascontiguousarray(kb.reshape(SC, 128).T),
                "idn": np.eye(128, dtype=np.float32).astype(ml_dtypes.bfloat16),
            }
        )
    return in_maps


def kernel(x, mask, Wq, Wk, Wv, Wo):
    nc = get_nc()
    in_maps = make_in_maps(x, mask, Wq, Wk, Wv, Wo)
    res = run_bass_kernel_spmd(nc, in_maps, list(range(N_CORES)))
    outs = np.empty((B, S, D), dtype=np.float32)
    for b in range(B):
        outs[b] = res.results[2 * b]["out"] + res.results[2 * b + 1]["out"]
    return outs


# revision 5
# speedup vs baseline: 1.4085x; 1.0217x over previous
"""Multi-head attention forward on 8 Trainium2 NeuronCores — v3.

Like v2 (AV in [q,d] orientation, bf16 Q/K/attn/Wo, per-partition normalize,
PE transposes) but with pipeline-aware emission for the in-order engines:

- Within a phase (head pair m, query tile qt), logits+exp for key chunk kc+2 are
  emitted BEFORE the AV matmuls of chunk kc, so the ACT engine always has two
  exp instructions in flight when an AV matmul blocks in the PE wait queue.
- The attn transposes of phase i are deferred into phase i+1 (through the cps
  PSUM ring, not the lg ring), so they never delay the next phase's logits.
- Stage-A projection chunks and stage-C output-projection groups drip one per
  key chunk through the cps ring, placed after the logits emission point.
- PSUM: lg [128,1024]x2 (4 banks), avA/avB [128,260]x1 (2), cps [128,512]x2 (2).
"""
import sys

sys.path.insert(0, "/opt/trn_rl_repo")

import numpy as np
from collections import deque

import concourse.bass as bass
import concourse.tile as tile
from concourse import mybir
from concourse.bass_utils import run_bass_kernel_spmd
from concourse.vector_clock import ScopedClock

_wsplit_ctr = [0]


def split_multi_waits(nc):
    """Walrus accepts at most one sync wait per instruction; split extras
    into single-wait nops."""
    for f in nc.m.functions:
        for bb in f.blocks:
            out = []
            changed = False
            for inst in bb.instructions:
                si = inst.sync_info
                waits = list(si.on_wait) if si is not None and si.on_wait else []
                if len(waits) > 1:
                    updates = list(si.on_update) if si.on_update else []
                    for w in waits[1:]:
                        _wsplit_ctr[0] += 1
                        nop = mybir.InstNoOp(
                            name=f"I-wsplit-{_wsplit_ctr[0]}", ins=[], outs=[]
                        )
                        nop.engine = inst.engine
                        nop.sync_info = mybir.SyncInfo(on_wait=[w], on_update=[])
                        out.append(nop)
                    inst.sync_info = mybir.SyncInfo(on_wait=[waits[0]], on_update=updates)
                    changed = True
                out.append(inst)
            if changed:
                bb.instructions = out
    return nc


B, S, D, H, DH = 4, 2048, 1024, 16, 64
HG = 2
LD = D // HG
LH = H // HG
N_CORES = B * HG
SCALE = float(DH) ** -0.5
NEG_INF = -1e30

FP = mybir.dt.float32
FPR = mybir.dt.float32r
BF = mybir.dt.bfloat16

KC = D // 128
MC = LD // 128
SC = S // 128
QT = S // 512
Exp = mybir.ActivationFunctionType.Exp
E1 = DH + 1


def _fr(ap):
    return ap.bitcast(FPR)


class SplitDrainTileContext(tile.TileContext):
    def _drain_and_barrier(self, tick_clock, wait_clock):
        nc = self.nc
        probe = nc.sync.nop()
        wait_clock.add_sem_waits(
            probe.ins, ScopedClock({None: tick_clock.global_clock})
        )
        si = probe.ins.sync_info
        waits = list(si.on_wait) if si is not None and si.on_wait else []
        updates = list(si.on_update) if si is not None and si.on_update else []
        if len(waits) > 1:
            probe.ins.sync_info = mybir.SyncInfo(on_wait=[waits[0]], on_update=updates)
            for w in waits[1:]:
                n2 = nc.sync.nop()
                n2.ins.sync_info = mybir.SyncInfo(on_wait=[w], on_update=[])
        nc.sync.drain()
        nc.all_engine_barrier()
        popped = nc._tile_sem_poison_stack.pop()
        assert popped is self._sem_poison
        nc.clear_and_free_semaphores(list(self.sems.allocated().values()))
        nc.all_engine_barrier()


def build_nc(for_hw=True):
    nc = bass.Bass(trn_type="TRN2")
    xT = nc.dram_tensor("xT", [D, S], BF, kind="ExternalInput").ap()
    wq = nc.dram_tensor("wq", [D, LD], BF, kind="ExternalInput").ap()
    wk = nc.dram_tensor("wk", [D, LD], BF, kind="ExternalInput").ap()
    wv = nc.dram_tensor("wv", [D, LD], BF, kind="ExternalInput").ap()
    wo = nc.dram_tensor("wo", [LD, D], BF, kind="ExternalInput").ap()
    kbias = nc.dram_tensor("kbias", [128, SC], FP, kind="ExternalInput").ap()
    idn = nc.dram_tensor("idn", [128, 128], BF, kind="ExternalInput").ap()
    out = nc.dram_tensor("out", [S, D], FP, kind="ExternalOutput").ap()

    with SplitDrainTileContext(nc) as tc:
        _body(tc, xT, wq, wk, wv, wo, kbias, idn, out)
    if for_hw:
        split_multi_waits(nc)
    return nc


def _body(tc, xT, wq, wk, wv, wo, kbias, idn, out):
    nc = tc.nc
    with (
        tc.tile_pool(name="pers", bufs=1) as pers,
        tc.tile_pool(name="pt", bufs=6) as pt_pool,
        tc.tile_pool(name="a2", bufs=8) as a2_pool,
        tc.tile_pool(name="rs", bufs=4) as rs_pool,
        tc.tile_pool(name="ot", bufs=6) as ot_pool,
        tc.tile_pool(name="psmm", bufs=1, space="PSUM") as psum,
    ):
        qt = [pers.tile([128, S], BF, tag=f"qt{m}", name=f"qt{m}") for m in range(MC)]
        kt = [pers.tile([128, S], BF, tag=f"kt{m}", name=f"kt{m}") for m in range(MC)]
        vt = [pers.tile([128, LH * E1], BF, tag=f"v{s}", name=f"v{s}") for s in range(SC)]
        attT = [pers.tile([128, S], BF, tag=f"at{m}", name=f"at{m}") for m in range(MC)]
        biasT = pers.tile([128, SC], FP, tag="biasT")
        idn_sb = pers.tile([128, 128], BF, tag="idn")

        for s in range(SC):
            dst = vt[s][:].rearrange("p (h e) -> p h e", e=E1)[:, :, DH : DH + 1]
            nc.vector.memset(dst, 1.0)

        # ---------- stage B phase emitter ----------
        def emit_B(m, qti, drip, rate=2):
            """One attention phase. Emits logits/exp two key-chunks ahead of
            the AV matmuls; pops drip closures after the logits point of
            every `rate`-th key chunk."""
            hA, hB = 2 * m, 2 * m + 1
            qs = slice(qti * 512, (qti + 1) * 512)
            avA = psum.tile([128, 4 * E1], FP, tag="avA", name="avA", bufs=1)
            avB = psum.tile([128, 4 * E1], FP, tag="avB", name="avB", bufs=1)
            pts = {}

            def logits(kc):
                ks = slice(kc * 128, (kc + 1) * 128)
                lg = psum.tile([128, 1024], FP, tag="lg", name="lg", bufs=2)
                nc.tensor.matmul(
                    lg[:, 0:512], kt[m][0:64, ks], qt[m][0:64, qs],
                    start=True, stop=True,
                )
                nc.tensor.matmul(
                    lg[:, 512:1024], kt[m][64:128, ks], qt[m][64:128, qs],
                    start=True, stop=True,
                )
                pt = pt_pool.tile([128, 1024], BF, tag="pt", name="pt")
                nc.scalar.activation(
                    pt[:], lg[:], Exp, bias=biasT[:, kc : kc + 1], scale=SCALE
                )
                pts[kc] = pt

            logits(0)
            logits(1)
            for kc in range(SC):
                pt = pts.pop(kc)
                for qc in range(4):
                    # start=True only on the first slice: the PSUM zero-region
                    # "pending zero" marking spans the whole bank, so later
                    # start=True calls would wipe sibling slices' first chunk
                    nc.tensor.matmul(
                        avA[:, qc * E1 : (qc + 1) * E1],
                        pt[:, qc * 128 : (qc + 1) * 128],
                        vt[kc][:, hA * E1 : (hA + 1) * E1],
                        start=(kc == 0 and qc == 0), stop=(kc == SC - 1),
                        skip_group_check=True,
                    )
                    nc.tensor.matmul(
                        avB[:, qc * E1 : (qc + 1) * E1],
                        pt[:, 512 + qc * 128 : 512 + (qc + 1) * 128],
                        vt[kc][:, hB * E1 : (hB + 1) * E1],
                        start=(kc == 0 and qc == 0), stop=(kc == SC - 1),
                        skip_group_check=True,
                    )
                if kc + 2 < SC:
                    logits(kc + 2)
                if drip and kc % rate == rate - 1 and (rate == 1 or kc < SC - 2):
                    drip.popleft()()
            # normalize into a2 staging tiles (bf16); transposes are deferred
            rsA = rs_pool.tile([128, 4], FP, tag="rs", name="rsA")
            rsB = rs_pool.tile([128, 4], FP, tag="rs", name="rsB")
            avAr = avA[:].rearrange("p (q e) -> p q e", e=E1)
            avBr = avB[:].rearrange("p (q e) -> p q e", e=E1)
            a2s = [a2_pool.tile([128, 128], BF, tag="a2", name="a2") for _ in range(4)]
            nc.vector.reciprocal(rsA[:], avAr[:, :, DH : DH + 1])
            for qc in range(4):
                nc.vector.tensor_scalar(
                    a2s[qc][:, 0:64], avA[:, qc * E1 : qc * E1 + DH],
                    rsA[:, qc : qc + 1], None, mybir.AluOpType.mult,
                )
            nc.vector.reciprocal(rsB[:], avBr[:, :, DH : DH + 1])
            for qc in range(4):
                nc.vector.tensor_scalar(
                    a2s[qc][:, 64:128], avB[:, qc * E1 : qc * E1 + DH],
                    rsB[:, qc : qc + 1], None, mybir.AluOpType.mult,
                )

            def tp_flush(m=m, qs=qs, a2s=a2s):
                tp = psum.tile([128, 512], BF, tag="cps", name="tp", bufs=2)
                for qc in range(4):
                    nc.tensor.matmul(
                        tp[:, qc * 128 : (qc + 1) * 128], a2s[qc][:], idn_sb[:],
                        is_transpose=True, skip_group_check=True,
                    )
                nc.vector.tensor_copy(attT[m][:, qs], tp[:])

            # leftover drips emit after the normalize so their copies never
            # delay the av-ring release at the phase boundary
            while drip:
                drip.popleft()()
            return tp_flush

        def c_group(qc, n, wo_all, tag="cps", eng=None):
            def emit():
                cps = psum.tile([128, 512], FP, tag=tag, name="cps", bufs=2)
                for j in range(MC):
                    nc.tensor.matmul(
                        cps[:],
                        attT[j][:, qc * 128 : (qc + 1) * 128],
                        wo_all[:, j * D + n * 512 : j * D + (n + 1) * 512],
                        start=(j == 0), stop=(j == MC - 1),
                    )
                ot = ot_pool.tile([128, 512], FP, tag="ot", name="ot")
                if eng == "act":
                    nc.scalar.copy(ot[:], cps[:])
                else:
                    nc.vector.tensor_copy(ot[:], cps[:])
                nc.sync.dma_start(
                    out[qc * 128 : (qc + 1) * 128, n * 512 : (n + 1) * 512],
                    ot[:],
                )

            return emit

        # ---------- stage A ----------
        with (
            tc.tile_pool(name="xt", bufs=1) as xt_pool,
            tc.tile_pool(name="w", bufs=1) as w_pool,
        ):
            # batched tiles: one DMA each (HWDGE gen is ~650ns per dma_start)
            wkm = [w_pool.tile([128, KC * 128], BF, tag=f"wkm{m}", name=f"wkm{m}") for m in range(MC)]
            wqm = [w_pool.tile([128, KC * 128], BF, tag=f"wqm{m}", name=f"wqm{m}") for m in range(MC)]
            wv_all = w_pool.tile([128, KC * LD], BF, tag="wv", name="wv_all")
            xs = [
                [xt_pool.tile([128, KC * 512], BF, tag=f"x{h}_{j}", name=f"x{h}_{j}") for j in range(2)]
                for h in range(2)
            ]
            wkR = wk.rearrange("(k r) c -> r k c", r=128)
            wqR = wq.rearrange("(k r) c -> r k c", r=128)
            wvR = wv.rearrange("(k r) c -> r k c", r=128)
            xR = xT.rearrange("(k r) q -> r k q", r=128)

            def xv(t):  # [128, (k c)] view
                return t[:].rearrange("r (k c) -> r k c", k=KC)

            # priority order: the tiny bias first (first exp needs it), then
            # inputs of the head chunks, split by contraction half so the
            # first accumulation chain starts early
            nc.sync.dma_start(biasT[:], kbias[:])
            for ks_ in (slice(0, 4), slice(4, 8)):
                nc.sync.dma_start(xv(wkm[0])[:, ks_], wkR[:, ks_, 0:128])
                nc.sync.dma_start(xv(xs[0][0])[:, ks_], xR[:, ks_, 0:512])
                nc.sync.dma_start(xv(wqm[0])[:, ks_], wqR[:, ks_, 0:128])
            nc.sync.dma_start(xv(wv_all), wvR[:, :, :])
            nc.sync.dma_start(xv(xs[0][1]), xR[:, :, 512:1024])
            for h, j in ((1, 0), (1, 1)):
                lo = h * 1024 + j * 512
                nc.sync.dma_start(xv(xs[h][j]), xR[:, :, lo : lo + 512])
            for m in range(1, MC):
                nc.sync.dma_start(xv(wkm[m]), wkR[:, :, m * 128 : (m + 1) * 128])
                nc.sync.dma_start(xv(wqm[m]), wqR[:, :, m * 128 : (m + 1) * 128])
            nc.sync.dma_start(idn_sb[:], idn[:])

            def qk_chunk(half, wtm, dstT, m, q2):
                ps = psum.tile([128, 512], FP, tag="cps", name="psa", bufs=2)
                wts = wtm[m]
                for k in range(KC):
                    nc.tensor.matmul(
                        ps[:],
                        wts[:, k * 128 : (k + 1) * 128],
                        xs[half][q2][:, k * 512 : (k + 1) * 512],
                        start=(k == 0), stop=(k == KC - 1),
                    )
                qlo = half * 1024 + q2 * 512
                nc.vector.tensor_copy(dstT[m][:, qlo : qlo + 512], ps[:])

            def v_chunk(s_idx):
                half, sc = s_idx // 8, s_idx % 8
                ps = psum.tile([128, 512], FP, tag="cps", name="psv", bufs=2)
                j, off = sc // 4, (sc % 4) * 128
                for k in range(KC):
                    nc.tensor.matmul(
                        ps[:],
                        xs[half][j][:, k * 512 + off : k * 512 + off + 128],
                        wv_all[:, k * LD : (k + 1) * LD],
                        start=(k == 0), stop=(k == KC - 1),
                    )
                src = ps[:].rearrange("p (h e) -> p h e", h=LH)
                dst = vt[s_idx][:].rearrange("p (h e) -> p h e", e=E1)[:, :, 0:DH]
                nc.vector.tensor_copy(dst, src)

            def A(fn, *args):
                return lambda: fn(*args)

            # minimal head: K(m0,h0,q2=0) covers logits kc0..3, Q(m0) first
            # query slice, V(s0..s3) covers the first AV chunks
            qk_chunk(0, wkm, kt, 0, 0)
            qk_chunk(0, wqm, qt, 0, 0)
            for s in range(4):
                v_chunk(s)

            wo_all = w_pool.tile([128, MC * D], BF, tag="wo", name="wo_all")
            nc.sync.dma_start(
                wo_all[:].rearrange("r (j c) -> r j c", j=MC),
                wo.rearrange("(j r) c -> r j c", r=128),
            )

            # (0,0) drip, rate 1: position p pops after logits(p+2); V_s must
            # sit at position <= s-1, K(m0,h*,q2) before logits emission of
            # its key range (kc4@slot2, kc8@slot6, kc12@slot10).
            d00 = [
                A(qk_chunk, 0, wkm, kt, 0, 1),  # keys 512:1024 (kc4+)
                A(v_chunk, 4), A(v_chunk, 5), A(v_chunk, 6),
                A(qk_chunk, 1, wkm, kt, 0, 0),  # keys 1024:1536 (kc8+)
                A(v_chunk, 7), A(v_chunk, 8), A(v_chunk, 9),
                A(qk_chunk, 1, wkm, kt, 0, 1),  # keys 1536:2048 (kc12+)
                A(v_chunk, 10), A(v_chunk, 11), A(v_chunk, 12), A(v_chunk, 13),
                A(v_chunk, 14), A(v_chunk, 15),
                A(qk_chunk, 0, wqm, qt, 0, 1),  # qt1 slice for (0,1)
            ]

            def C(qti, lo, hi, tag="cps"):  # out-proj groups [lo,hi) of tile qti
                return [c_group(qc, n, wo_all, tag)
                        for g in range(lo, hi)
                        for qc, n in [(4 * qti + g // 2, g % 2)]]

            K_ = lambda m, h, q2: A(qk_chunk, h, wkm, kt, m, q2)
            Q_ = lambda m, h, q2: A(qk_chunk, h, wqm, qt, m, q2)
            # anti-diagonal phase rotation: stage-A chunks spread so nearly
            # every phase stays ACT(exp)-paced; None marks where the previous
            # phase's deferred transpose-flush goes.
            rotation = [
                ((0, 1), [None, K_(1, 0, 0), Q_(1, 0, 0), Q_(0, 1, 0)]),
                ((1, 0), [K_(1, 0, 1), K_(1, 1, 0), K_(1, 1, 1), None, Q_(0, 1, 1)]),
                ((0, 2), [None, Q_(1, 0, 1), K_(2, 0, 0)]),
                ((1, 1), [None, Q_(2, 0, 0), Q_(1, 1, 0)]),
                ((2, 0), [K_(2, 0, 1), K_(2, 1, 0), K_(2, 1, 1), None]),
                ((0, 3), [None, Q_(2, 0, 1)]),
                ((1, 2), [None, K_(3, 0, 0), Q_(3, 0, 0)]),
                ((2, 1), [None, Q_(1, 1, 1)]),
                ((3, 0), [K_(3, 0, 1), K_(3, 1, 0), K_(3, 1, 1), None, Q_(2, 1, 0)]),
                ((1, 3), [None, Q_(3, 0, 1)] + C(0, 0, 2)),
                ((2, 2), [None, Q_(2, 1, 1)] + C(0, 2, 6)),
                ((3, 1), [None, Q_(3, 1, 0)] + C(0, 6, 8)),
                ((3, 2), [None, Q_(3, 1, 1)] + C(1, 0, 5)),
                ((2, 3), [None] + C(1, 5, 8)),
                ((3, 3), [None] + C(2, 0, 8)),
            ]

            drip = deque(d00)
            tpf = emit_B(0, 0, drip, rate=1)
            for (m, qti), items in rotation:
                i = items.index(None)
                drip.extend(items[:i])
                drip.append(tpf)
                drip.extend(items[i + 1 :])
                tpf = emit_B(m, qti, drip)
            # tail: last transpose-flush + last query tile's output projection,
            # alternating PSUM rings (lg ring is free after the last exp)
            tpf()
            for i in range(8):
                c_group(12 + i // 2, i % 2, wo_all, "lg" if i % 2 else "cps",
                        eng="act" if i % 2 else None)()


_nc = None


def get_nc():
    global _nc
    if _nc is None:
        _nc = build_nc()
    return _nc


def make_in_maps(x, mask, Wq, Wk, Wv, Wo):
    import ml_dtypes

    x = np.asarray(x, dtype=np.float32)
    mask = np.asarray(mask)
    Wq, Wk, Wv, Wo = (np.asarray(w, dtype=np.float32) for w in (Wq, Wk, Wv, Wo))
    in_maps = []
    for c in range(N_CORES):
        b, hg = c // HG, c % HG
        lo, hi = hg * LD, (hg + 1) * LD
        kb = np.where(mask[b], 0.0, NEG_INF).astype(np.float32)
        in_maps.append(
            {
                "xT": np.ascontiguousarray(x[b].T).astype(ml_dtypes.bfloat16),
                "wq": np.ascontiguousarray(Wq[:, lo:hi]).astype(ml_dtypes.bfloat16),
                "wk": np.ascontiguousarray(Wk[:, lo:hi]).astype(ml_dtypes.bfloat16),
                "wv": np.ascontiguousarray(Wv[:, lo:hi]).astype(ml_dtypes.bfloat16),
                "wo": np.ascontiguousarray(Wo[lo:hi, :]).astype(ml_dtypes.bfloat16),
                "kbias": np.ascontiguousarray(kb.reshape(SC, 128).T),
                "idn": np.eye(128, dtype=np.float32).astype(ml_dtypes.bfloat16),
            }
        )
    return in_maps


def kernel(x, mask, Wq, Wk, Wv, Wo):
    nc = get_nc()
    in_maps = make_in_maps(x, mask, Wq, Wk, Wv, Wo)
    res = run_bass_kernel_spmd(nc, in_maps, list(range(N_CORES)))
    outs = np.empty((B, S, D), dtype=np.float32)
    for b in range(B):
        outs[b] = res.results[2 * b]["out"] + res.results[2 * b + 1]["out"]
    return outs


# revision 6
# speedup vs baseline: 1.4566x; 1.0341x over previous
"""Multi-head attention forward on 8 Trainium2 NeuronCores — v3.

Like v2 (AV in [q,d] orientation, bf16 Q/K/attn/Wo, per-partition normalize,
PE transposes) but with pipeline-aware emission for the in-order engines:

- Within a phase (head pair m, query tile qt), logits+exp for key chunk kc+2 are
  emitted BEFORE the AV matmuls of chunk kc, so the ACT engine always has two
  exp instructions in flight when an AV matmul blocks in the PE wait queue.
- The attn transposes of phase i are deferred into phase i+1 (through the cps
  PSUM ring, not the lg ring), so they never delay the next phase's logits.
- Stage-A projection chunks and stage-C output-projection groups drip one per
  key chunk through the cps ring, placed after the logits emission point.
- PSUM: lg [128,1024]x2 (4 banks), avA/avB [128,260]x1 (2), cps [128,512]x2 (2).
"""
import sys

sys.path.insert(0, "/opt/trn_rl_repo")

import numpy as np
from collections import deque

import concourse.bass as bass
import concourse.tile as tile
from concourse import mybir
from concourse.bass_utils import run_bass_kernel_spmd
from concourse.vector_clock import ScopedClock

_wsplit_ctr = [0]


def split_multi_waits(nc):
    """Walrus accepts at most one sync wait per instruction; split extras
    into single-wait nops."""
    for f in nc.m.functions:
        for bb in f.blocks:
            out = []
            changed = False
            for inst in bb.instructions:
                si = inst.sync_info
                waits = list(si.on_wait) if si is not None and si.on_wait else []
                if len(waits) > 1:
                    updates = list(si.on_update) if si.on_update else []
                    for w in waits[1:]:
                        _wsplit_ctr[0] += 1
                        nop = mybir.InstNoOp(
                            name=f"I-wsplit-{_wsplit_ctr[0]}", ins=[], outs=[]
                        )
                        nop.engine = inst.engine
                        nop.sync_info = mybir.SyncInfo(on_wait=[w], on_update=[])
                        out.append(nop)
                    inst.sync_info = mybir.SyncInfo(on_wait=[waits[0]], on_update=updates)
                    changed = True
                out.append(inst)
            if changed:
                bb.instructions = out
    return nc


B, S, D, H, DH = 4, 2048, 1024, 16, 64
HG = 2
LD = D // HG
LH = H // HG
N_CORES = B * HG
SCALE = float(DH) ** -0.5
NEG_INF = -1e30

FP = mybir.dt.float32
FPR = mybir.dt.float32r
BF = mybir.dt.bfloat16

KC = D // 128
MC = LD // 128
SC = S // 128
QT = S // 512
Exp = mybir.ActivationFunctionType.Exp
E1 = DH + 1


def _fr(ap):
    return ap.bitcast(FPR)


class SplitDrainTileContext(tile.TileContext):
    def _drain_and_barrier(self, tick_clock, wait_clock):
        nc = self.nc
        probe = nc.sync.nop()
        wait_clock.add_sem_waits(
            probe.ins, ScopedClock({None: tick_clock.global_clock})
        )
        si = probe.ins.sync_info
        waits = list(si.on_wait) if si is not None and si.on_wait else []
        updates = list(si.on_update) if si is not None and si.on_update else []
        if len(waits) > 1:
            probe.ins.sync_info = mybir.SyncInfo(on_wait=[waits[0]], on_update=updates)
            for w in waits[1:]:
                n2 = nc.sync.nop()
                n2.ins.sync_info = mybir.SyncInfo(on_wait=[w], on_update=[])
        nc.sync.drain()
        nc.all_engine_barrier()
        popped = nc._tile_sem_poison_stack.pop()
        assert popped is self._sem_poison
        nc.clear_and_free_semaphores(list(self.sems.allocated().values()))
        nc.all_engine_barrier()


def build_nc(for_hw=True):
    nc = bass.Bass(trn_type="TRN2")
    xT = nc.dram_tensor("xT", [D, S], BF, kind="ExternalInput").ap()
    wq = nc.dram_tensor("wq", [D, LD], BF, kind="ExternalInput").ap()
    wk = nc.dram_tensor("wk", [D, LD], BF, kind="ExternalInput").ap()
    wv = nc.dram_tensor("wv", [D, LD], BF, kind="ExternalInput").ap()
    wo = nc.dram_tensor("wo", [LD, D], BF, kind="ExternalInput").ap()
    kbias = nc.dram_tensor("kbias", [128, SC], FP, kind="ExternalInput").ap()
    idn = nc.dram_tensor("idn", [128, 128], BF, kind="ExternalInput").ap()
    out = nc.dram_tensor("out", [S, D], FP, kind="ExternalOutput").ap()

    with SplitDrainTileContext(nc) as tc:
        _body(tc, xT, wq, wk, wv, wo, kbias, idn, out)
    if for_hw:
        split_multi_waits(nc)
    return nc


def _body(tc, xT, wq, wk, wv, wo, kbias, idn, out):
    nc = tc.nc
    with (
        tc.tile_pool(name="pers", bufs=1) as pers,
        tc.tile_pool(name="pt", bufs=6) as pt_pool,
        tc.tile_pool(name="a2", bufs=8) as a2_pool,
        tc.tile_pool(name="rs", bufs=4) as rs_pool,
        tc.tile_pool(name="ot", bufs=6) as ot_pool,
        tc.tile_pool(name="psmm", bufs=1, space="PSUM") as psum,
    ):
        qt = [pers.tile([128, S], BF, tag=f"qt{m}", name=f"qt{m}") for m in range(MC)]
        kt = [pers.tile([128, S], BF, tag=f"kt{m}", name=f"kt{m}") for m in range(MC)]
        vt = [pers.tile([128, LH * E1], BF, tag=f"v{s}", name=f"v{s}") for s in range(SC)]
        attT = [pers.tile([128, S], BF, tag=f"at{m}", name=f"at{m}") for m in range(MC)]
        biasT = pers.tile([128, SC], FP, tag="biasT")
        idn_sb = pers.tile([128, 128], BF, tag="idn")

        for s in range(SC):
            dst = vt[s][:].rearrange("p (h e) -> p h e", e=E1)[:, :, DH : DH + 1]
            nc.vector.memset(dst, 1.0)

        # ---------- stage B phase emitter ----------
        def emit_B(m, qti, drip, rate=2):
            """One attention phase. Emits logits/exp two key-chunks ahead of
            the AV matmuls; pops drip closures after the logits point of
            every `rate`-th key chunk."""
            hA, hB = 2 * m, 2 * m + 1
            qs = slice(qti * 512, (qti + 1) * 512)
            avA = psum.tile([128, 4 * E1], FP, tag="avA", name="avA", bufs=1)
            avB = psum.tile([128, 4 * E1], FP, tag="avB", name="avB", bufs=1)
            pts = {}

            def logits(kc):
                ks = slice(kc * 128, (kc + 1) * 128)
                lg = psum.tile([128, 1024], FP, tag="lg", name="lg", bufs=2)
                nc.tensor.matmul(
                    lg[:, 0:512], kt[m][0:64, ks], qt[m][0:64, qs],
                    start=True, stop=True,
                )
                nc.tensor.matmul(
                    lg[:, 512:1024], kt[m][64:128, ks], qt[m][64:128, qs],
                    start=True, stop=True,
                )
                pt = pt_pool.tile([128, 1024], BF, tag="pt", name="pt")
                nc.scalar.activation(
                    pt[:], lg[:], Exp, bias=biasT[:, kc : kc + 1], scale=SCALE
                )
                pts[kc] = pt

            logits(0)
            logits(1)
            for kc in range(SC):
                pt = pts.pop(kc)
                for qc in range(4):
                    # start=True only on the first slice: the PSUM zero-region
                    # "pending zero" marking spans the whole bank, so later
                    # start=True calls would wipe sibling slices' first chunk
                    nc.tensor.matmul(
                        avA[:, qc * E1 : (qc + 1) * E1],
                        pt[:, qc * 128 : (qc + 1) * 128],
                        vt[kc][:, hA * E1 : (hA + 1) * E1],
                        start=(kc == 0 and qc == 0), stop=(kc == SC - 1),
                        skip_group_check=True,
                    )
                    nc.tensor.matmul(
                        avB[:, qc * E1 : (qc + 1) * E1],
                        pt[:, 512 + qc * 128 : 512 + (qc + 1) * 128],
                        vt[kc][:, hB * E1 : (hB + 1) * E1],
                        start=(kc == 0 and qc == 0), stop=(kc == SC - 1),
                        skip_group_check=True,
                    )
                if kc + 2 < SC:
                    logits(kc + 2)
                if drip and kc % rate == rate - 1 and (rate == 1 or kc < SC - 2):
                    drip.popleft()()
            # normalize into a2 staging tiles (bf16); transposes are deferred
            rsA = rs_pool.tile([128, 4], FP, tag="rs", name="rsA")
            rsB = rs_pool.tile([128, 4], FP, tag="rs", name="rsB")
            avAr = avA[:].rearrange("p (q e) -> p q e", e=E1)
            avBr = avB[:].rearrange("p (q e) -> p q e", e=E1)
            a2s = [a2_pool.tile([128, 128], BF, tag="a2", name="a2") for _ in range(4)]
            nc.vector.reciprocal(rsA[:], avAr[:, :, DH : DH + 1])
            for qc in range(4):
                nc.vector.tensor_scalar(
                    a2s[qc][:, 0:64], avA[:, qc * E1 : qc * E1 + DH],
                    rsA[:, qc : qc + 1], None, mybir.AluOpType.mult,
                )
            nc.vector.reciprocal(rsB[:], avBr[:, :, DH : DH + 1])
            for qc in range(4):
                nc.vector.tensor_scalar(
                    a2s[qc][:, 64:128], avB[:, qc * E1 : qc * E1 + DH],
                    rsB[:, qc : qc + 1], None, mybir.AluOpType.mult,
                )

            def tp_flush(m=m, qs=qs, a2s=a2s):
                tp = psum.tile([128, 512], BF, tag="cps", name="tp", bufs=2)
                for qc in range(4):
                    nc.tensor.matmul(
                        tp[:, qc * 128 : (qc + 1) * 128], a2s[qc][:], idn_sb[:],
                        is_transpose=True, skip_group_check=True,
                    )
                nc.vector.tensor_copy(attT[m][:, qs], tp[:])

            # leftover drips emit after the normalize so their copies never
            # delay the av-ring release at the phase boundary
            while drip:
                drip.popleft()()
            return tp_flush

        def c_group(qc, n, wo_all, tag="cps", eng=None):
            def emit():
                cps = psum.tile([128, 512], FP, tag=tag, name="cps", bufs=2)
                for j in range(MC):
                    nc.tensor.matmul(
                        cps[:],
                        attT[j][:, qc * 128 : (qc + 1) * 128],
                        wo_all[:, j * D + n * 512 : j * D + (n + 1) * 512],
                        start=(j == 0), stop=(j == MC - 1),
                    )
                ot = ot_pool.tile([128, 512], FP, tag="ot", name="ot")
                if eng == "act":
                    nc.scalar.copy(ot[:], cps[:])
                else:
                    nc.vector.tensor_copy(ot[:], cps[:])
                nc.sync.dma_start(
                    out[qc * 128 : (qc + 1) * 128, n * 512 : (n + 1) * 512],
                    ot[:],
                )

            return emit

        # ---------- stage A ----------
        with (
            tc.tile_pool(name="xt", bufs=1) as xt_pool,
            tc.tile_pool(name="w", bufs=1) as w_pool,
        ):
            # batched tiles: one DMA each (HWDGE gen is ~650ns per dma_start)
            wkm = [w_pool.tile([128, KC * 128], BF, tag=f"wkm{m}", name=f"wkm{m}") for m in range(MC)]
            wqm = [w_pool.tile([128, KC * 128], BF, tag=f"wqm{m}", name=f"wqm{m}") for m in range(MC)]
            wv_all = w_pool.tile([128, KC * LD], BF, tag="wv", name="wv_all")
            xs = [
                [xt_pool.tile([128, KC * 512], BF, tag=f"x{h}_{j}", name=f"x{h}_{j}") for j in range(2)]
                for h in range(2)
            ]
            wkR = wk.rearrange("(k r) c -> r k c", r=128)
            wqR = wq.rearrange("(k r) c -> r k c", r=128)
            wvR = wv.rearrange("(k r) c -> r k c", r=128)
            xR = xT.rearrange("(k r) q -> r k q", r=128)

            def xv(t):  # [128, (k c)] view
                return t[:].rearrange("r (k c) -> r k c", k=KC)

            # priority order: the tiny bias first (first exp needs it), then
            # inputs of the head chunks, split by contraction half so the
            # first accumulation chain starts early
            nc.sync.dma_start(biasT[:], kbias[:])
            for ks_ in (slice(0, 4), slice(4, 8)):
                nc.sync.dma_start(xv(wkm[0])[:, ks_], wkR[:, ks_, 0:128])
                nc.sync.dma_start(xv(xs[0][0])[:, ks_], xR[:, ks_, 0:512])
                nc.sync.dma_start(xv(wqm[0])[:, ks_], wqR[:, ks_, 0:128])
            nc.sync.dma_start(xv(wv_all), wvR[:, :, :])
            nc.sync.dma_start(xv(xs[0][1]), xR[:, :, 512:1024])
            for h, j in ((1, 0), (1, 1)):
                lo = h * 1024 + j * 512
                nc.sync.dma_start(xv(xs[h][j]), xR[:, :, lo : lo + 512])
            for m in range(1, MC):
                nc.sync.dma_start(xv(wkm[m]), wkR[:, :, m * 128 : (m + 1) * 128])
                nc.sync.dma_start(xv(wqm[m]), wqR[:, :, m * 128 : (m + 1) * 128])
            nc.sync.dma_start(idn_sb[:], idn[:])

            def qk_chunk(half, wtm, dstT, m, q2):
                ps = psum.tile([128, 512], FP, tag="cps", name="psa", bufs=2)
                wts = wtm[m]
                for k in range(KC):
                    nc.tensor.matmul(
                        ps[:],
                        wts[:, k * 128 : (k + 1) * 128],
                        xs[half][q2][:, k * 512 : (k + 1) * 512],
                        start=(k == 0), stop=(k == KC - 1),
                    )
                qlo = half * 1024 + q2 * 512
                nc.vector.tensor_copy(dstT[m][:, qlo : qlo + 512], ps[:])

            def v_chunk(s_idx, m):
                # V columns of head pair m only: the early window needs just
                # pair 0; the rest drips into later phases' slack
                half, sc = s_idx // 8, s_idx % 8
                ps = psum.tile([128, 128], FP, tag="cps", name="psv", bufs=2)
                j, off = sc // 4, (sc % 4) * 128
                for k in range(KC):
                    nc.tensor.matmul(
                        ps[:],
                        xs[half][j][:, k * 512 + off : k * 512 + off + 128],
                        wv_all[:, k * LD + m * 128 : k * LD + (m + 1) * 128],
                        start=(k == 0), stop=(k == KC - 1),
                    )
                src = ps[:].rearrange("p (h e) -> p h e", h=2)
                dst = vt[s_idx][:].rearrange("p (h e) -> p h e", e=E1)[
                    :, 2 * m : 2 * m + 2, 0:DH
                ]
                nc.vector.tensor_copy(dst, src)

            def A(fn, *args):
                return lambda: fn(*args)

            # minimal head: K(m0,h0,q2=0) covers logits kc0..3, Q(m0) first
            # query slice, V(s0..s3) covers the first AV chunks
            qk_chunk(0, wkm, kt, 0, 0)
            qk_chunk(0, wqm, qt, 0, 0)
            for s in range(4):
                v_chunk(s, 0)

            wo_all = w_pool.tile([128, MC * D], BF, tag="wo", name="wo_all")
            nc.sync.dma_start(
                wo_all[:].rearrange("r (j c) -> r j c", j=MC),
                wo.rearrange("(j r) c -> r j c", r=128),
            )

            # (0,0) drip, rate 1: position p pops after logits(p+2); V_s must
            # sit at position <= s-1, K(m0,h*,q2) before logits emission of
            # its key range (kc4@slot2, kc8@slot6, kc12@slot10).
            d00 = [
                A(qk_chunk, 0, wkm, kt, 0, 1),  # keys 512:1024 (kc4+)
                A(v_chunk, 4, 0), A(v_chunk, 5, 0), A(v_chunk, 6, 0),
                A(qk_chunk, 1, wkm, kt, 0, 0),  # keys 1024:1536 (kc8+)
                A(v_chunk, 7, 0), A(v_chunk, 8, 0), A(v_chunk, 9, 0),
                A(qk_chunk, 1, wkm, kt, 0, 1),  # keys 1536:2048 (kc12+)
                A(v_chunk, 10, 0), A(v_chunk, 11, 0), A(v_chunk, 12, 0),
                A(v_chunk, 13, 0), A(v_chunk, 14, 0), A(v_chunk, 15, 0),
                A(qk_chunk, 0, wqm, qt, 0, 1),  # qt1 slice for (0,1)
            ]
            V_ = lambda s, m: A(v_chunk, s, m)

            def C(qti, lo, hi, tag="cps"):  # out-proj groups [lo,hi) of tile qti
                return [c_group(qc, n, wo_all, tag)
                        for g in range(lo, hi)
                        for qc, n in [(4 * qti + g // 2, g % 2)]]

            K_ = lambda m, h, q2: A(qk_chunk, h, wkm, kt, m, q2)
            Q_ = lambda m, h, q2: A(qk_chunk, h, wqm, qt, m, q2)
            # anti-diagonal phase rotation: stage-A chunks spread so nearly
            # every phase stays ACT(exp)-paced; None marks where the previous
            # phase's deferred transpose-flush goes.
            rotation = [
                ((0, 1), 1, [None, K_(1, 0, 0), Q_(1, 0, 0), Q_(0, 1, 0)]
                         + [V_(s, 1) for s in range(0, 12)]),
                ((1, 0), 1, [K_(1, 0, 1), V_(12, 1), K_(1, 1, 0), V_(13, 1),
                             K_(1, 1, 1), V_(14, 1), V_(15, 1), None, Q_(0, 1, 1)]),
                ((0, 2), 2, [None, Q_(1, 0, 1), K_(2, 0, 0)]
                         + [V_(s, 2) for s in range(0, 4)]),
                ((1, 1), 2, [None, Q_(2, 0, 0), Q_(1, 1, 0)]
                         + [V_(s, 2) for s in range(4, 8)]),
                ((2, 0), 1, [K_(2, 0, 1), V_(8, 2), K_(2, 1, 0), V_(9, 2),
                             K_(2, 1, 1), None]
                         + [V_(s, 2) for s in range(10, 16)]),
                ((0, 3), 2, [None, Q_(2, 0, 1)] + [V_(s, 3) for s in range(0, 6)]),
                ((1, 2), 2, [None, K_(3, 0, 0), Q_(3, 0, 0)]
                         + [V_(s, 3) for s in range(6, 10)]),
                ((2, 1), 2, [None, Q_(1, 1, 1)] + [V_(s, 3) for s in range(10, 14)]),
                ((3, 0), 2, [K_(3, 0, 1), V_(14, 3), K_(3, 1, 0), V_(15, 3),
                             K_(3, 1, 1), None, Q_(2, 1, 0)]),
                ((1, 3), 2, [None, Q_(3, 0, 1)] + C(0, 0, 2)),
                ((2, 2), 2, [None, Q_(2, 1, 1)] + C(0, 2, 6)),
                ((3, 1), 2, [None, Q_(3, 1, 0)] + C(0, 6, 8)),
                ((3, 2), 2, [None, Q_(3, 1, 1)] + C(1, 0, 5)),
                ((2, 3), 2, [None] + C(1, 5, 8)),
                ((3, 3), 2, [None] + C(2, 0, 8)),
            ]

            drip = deque(d00)
            tpf = emit_B(0, 0, drip, rate=1)
            for (m, qti), rate_, items in rotation:
                i = items.index(None)
                drip.extend(items[:i])
                drip.append(tpf)
                drip.extend(items[i + 1 :])
                tpf = emit_B(m, qti, drip, rate=rate_)
            # tail: last transpose-flush + last query tile's output projection,
            # alternating PSUM rings (lg ring is free after the last exp)
            tpf()
            for i in range(8):
                c_group(12 + i // 2, i % 2, wo_all, "lg" if i % 2 else "cps",
                        eng="act" if i % 2 else None)()


_nc = None


def get_nc():
    global _nc
    if _nc is None:
        _nc = build_nc()
    return _nc


def make_in_maps(x, mask, Wq, Wk, Wv, Wo):
    import ml_dtypes

    x = np.asarray(x, dtype=np.float32)
    mask = np.asarray(mask)
    Wq, Wk, Wv, Wo = (np.asarray(w, dtype=np.float32) for w in (Wq, Wk, Wv, Wo))
    in_maps = []
    for c in range(N_CORES):
        b, hg = c // HG, c % HG
        lo, hi = hg * LD, (hg + 1) * LD
        kb = np.where(mask[b], 0.0, NEG_INF).astype(np.float32)
        in_maps.append(
            {
                "xT": np.ascontiguousarray(x[b].T).astype(ml_dtypes.bfloat16),
                "wq": np.ascontiguousarray(Wq[:, lo:hi]).astype(ml_dtypes.bfloat16),
                "wk": np.ascontiguousarray(Wk[:, lo:hi]).astype(ml_dtypes.bfloat16),
                "wv": np.ascontiguousarray(Wv[:, lo:hi]).astype(ml_dtypes.bfloat16),
                "wo": np.ascontiguousarray(Wo[lo:hi, :]).astype(ml_dtypes.bfloat16),
                "kbias": np.ascontiguousarray(kb.reshape(SC, 128).T),
                "idn": np.eye(128, dtype=np.float32).astype(ml_dtypes.bfloat16),
            }
        )
    return in_maps


def kernel(x, mask, Wq, Wk, Wv, Wo):
    nc = get_nc()
    in_maps = make_in_maps(x, mask, Wq, Wk, Wv, Wo)
    res = run_bass_kernel_spmd(nc, in_maps, list(range(N_CORES)))
    outs = np.empty((B, S, D), dtype=np.float32)
    for b in range(B):
        outs[b] = res.results[2 * b]["out"] + res.results[2 * b + 1]["out"]
    return outs


# revision 8
# speedup vs baseline: 1.4963x; 1.0273x over previous
"""Multi-head attention forward on 8 Trainium2 NeuronCores — v3.

Like v2 (AV in [q,d] orientation, bf16 Q/K/attn/Wo, per-partition normalize,
PE transposes) but with pipeline-aware emission for the in-order engines:

- Within a phase (head pair m, query tile qt), logits+exp for key chunk kc+2 are
  emitted BEFORE the AV matmuls of chunk kc, so the ACT engine always has two
  exp instructions in flight when an AV matmul blocks in the PE wait queue.
- The attn transposes of phase i are deferred into phase i+1 (through the cps
  PSUM ring, not the lg ring), so they never delay the next phase's logits.
- Stage-A projection chunks and stage-C output-projection groups drip one per
  key chunk through the cps ring, placed after the logits emission point.
- PSUM: lg [128,1024]x2 (4 banks), avA/avB [128,260]x1 (2), cps [128,512]x2 (2).
"""
import sys

sys.path.insert(0, "/opt/trn_rl_repo")

import numpy as np
from collections import deque

import concourse.bass as bass
import concourse.tile as tile
from concourse import mybir
from concourse.bass_utils import run_bass_kernel_spmd
from concourse.vector_clock import ScopedClock

_wsplit_ctr = [0]


def split_multi_waits(nc):
    """Walrus accepts at most one sync wait per instruction; split extras
    into single-wait nops."""
    for f in nc.m.functions:
        for bb in f.blocks:
            out = []
            changed = False
            for inst in bb.instructions:
                si = inst.sync_info
                waits = list(si.on_wait) if si is not None and si.on_wait else []
                if len(waits) > 1:
                    updates = list(si.on_update) if si.on_update else []
                    for w in waits[1:]:
                        _wsplit_ctr[0] += 1
                        nop = mybir.InstNoOp(
                            name=f"I-wsplit-{_wsplit_ctr[0]}", ins=[], outs=[]
                        )
                        nop.engine = inst.engine
                        nop.sync_info = mybir.SyncInfo(on_wait=[w], on_update=[])
                        out.append(nop)
                    inst.sync_info = mybir.SyncInfo(on_wait=[waits[0]], on_update=updates)
                    changed = True
                out.append(inst)
            if changed:
                bb.instructions = out
    return nc


B, S, D, H, DH = 4, 2048, 1024, 16, 64
HG = 2
LD = D // HG
LH = H // HG
N_CORES = B * HG
SCALE = float(DH) ** -0.5
NEG_INF = -1e30

FP = mybir.dt.float32
FPR = mybir.dt.float32r
BF = mybir.dt.bfloat16

KC = D // 128
MC = LD // 128
SC = S // 128
QT = S // 512
Exp = mybir.ActivationFunctionType.Exp
E1 = DH + 1


def _fr(ap):
    return ap.bitcast(FPR)


class SplitDrainTileContext(tile.TileContext):
    def _drain_and_barrier(self, tick_clock, wait_clock):
        nc = self.nc
        probe = nc.sync.nop()
        wait_clock.add_sem_waits(
            probe.ins, ScopedClock({None: tick_clock.global_clock})
        )
        si = probe.ins.sync_info
        waits = list(si.on_wait) if si is not None and si.on_wait else []
        updates = list(si.on_update) if si is not None and si.on_update else []
        if len(waits) > 1:
            probe.ins.sync_info = mybir.SyncInfo(on_wait=[waits[0]], on_update=updates)
            for w in waits[1:]:
                n2 = nc.sync.nop()
                n2.ins.sync_info = mybir.SyncInfo(on_wait=[w], on_update=[])
        nc.sync.drain()
        nc.all_engine_barrier()
        popped = nc._tile_sem_poison_stack.pop()
        assert popped is self._sem_poison
        nc.clear_and_free_semaphores(list(self.sems.allocated().values()))
        nc.all_engine_barrier()


def build_nc(for_hw=True):
    nc = bass.Bass(trn_type="TRN2")
    xT = nc.dram_tensor("xT", [D, S], BF, kind="ExternalInput").ap()
    wq = nc.dram_tensor("wq", [D, LD], BF, kind="ExternalInput").ap()
    wk = nc.dram_tensor("wk", [D, LD], BF, kind="ExternalInput").ap()
    wv = nc.dram_tensor("wv", [D, LD], BF, kind="ExternalInput").ap()
    wo = nc.dram_tensor("wo", [LD, D], BF, kind="ExternalInput").ap()
    kbias = nc.dram_tensor("kbias", [128, SC], FP, kind="ExternalInput").ap()
    idn = nc.dram_tensor("idn", [128, 128], BF, kind="ExternalInput").ap()
    out = nc.dram_tensor("out", [S, D], FP, kind="ExternalOutput").ap()

    with SplitDrainTileContext(nc) as tc:
        _body(tc, xT, wq, wk, wv, wo, kbias, idn, out)
    if for_hw:
        split_multi_waits(nc)
    return nc


def _body(tc, xT, wq, wk, wv, wo, kbias, idn, out):
    nc = tc.nc
    with (
        tc.tile_pool(name="pers", bufs=1) as pers,
        tc.tile_pool(name="pt", bufs=6) as pt_pool,
        tc.tile_pool(name="a2", bufs=8) as a2_pool,
        tc.tile_pool(name="rs", bufs=4) as rs_pool,
        tc.tile_pool(name="ot", bufs=6) as ot_pool,
        tc.tile_pool(name="psmm", bufs=1, space="PSUM") as psum,
    ):
        qt = [pers.tile([128, S], BF, tag=f"qt{m}", name=f"qt{m}") for m in range(MC)]
        kt = [pers.tile([128, S], BF, tag=f"kt{m}", name=f"kt{m}") for m in range(MC)]
        vt = [pers.tile([128, LH * E1], BF, tag=f"v{s}", name=f"v{s}") for s in range(SC)]
        attT = [pers.tile([128, S], BF, tag=f"at{m}", name=f"at{m}") for m in range(MC)]
        biasT = pers.tile([128, SC], FP, tag="biasT")
        idn_sb = pers.tile([128, 128], BF, tag="idn")

        for s in range(SC):
            dst = vt[s][:].rearrange("p (h e) -> p h e", e=E1)[:, :, DH : DH + 1]
            nc.vector.memset(dst, 1.0)

        # ---------- stage B phase emitter ----------
        def emit_B(m, qti, drip, rate=2):
            """One attention phase. Emits logits/exp two key-chunks ahead of
            the AV matmuls; pops drip closures after the logits point of
            every `rate`-th key chunk."""
            hA, hB = 2 * m, 2 * m + 1
            qs = slice(qti * 512, (qti + 1) * 512)
            avA = psum.tile([128, 4 * E1], FP, tag="avA", name="avA", bufs=1)
            avB = psum.tile([128, 4 * E1], FP, tag="avB", name="avB", bufs=1)
            pts = {}

            def logits(kc):
                ks = slice(kc * 128, (kc + 1) * 128)
                lg = psum.tile([128, 1024], FP, tag="lg", name="lg", bufs=2)
                nc.tensor.matmul(
                    lg[:, 0:512], kt[m][0:64, ks], qt[m][0:64, qs],
                    start=True, stop=True,
                )
                nc.tensor.matmul(
                    lg[:, 512:1024], kt[m][64:128, ks], qt[m][64:128, qs],
                    start=True, stop=True,
                )
                pt = pt_pool.tile([128, 1024], BF, tag="pt", name="pt")
                nc.scalar.activation(
                    pt[:], lg[:], Exp, bias=biasT[:, kc : kc + 1], scale=SCALE
                )
                pts[kc] = pt

            logits(0)
            logits(1)
            for kc in range(SC):
                pt = pts.pop(kc)
                for qc in range(4):
                    # start=True only on the first slice: the PSUM zero-region
                    # "pending zero" marking spans the whole bank, so later
                    # start=True calls would wipe sibling slices' first chunk
                    nc.tensor.matmul(
                        avA[:, qc * E1 : (qc + 1) * E1],
                        pt[:, qc * 128 : (qc + 1) * 128],
                        vt[kc][:, hA * E1 : (hA + 1) * E1],
                        start=(kc == 0 and qc == 0), stop=(kc == SC - 1),
                        skip_group_check=True,
                    )
                    nc.tensor.matmul(
                        avB[:, qc * E1 : (qc + 1) * E1],
                        pt[:, 512 + qc * 128 : 512 + (qc + 1) * 128],
                        vt[kc][:, hB * E1 : (hB + 1) * E1],
                        start=(kc == 0 and qc == 0), stop=(kc == SC - 1),
                        skip_group_check=True,
                    )
                if kc + 2 < SC:
                    logits(kc + 2)
                if drip and kc % rate == rate - 1 and (rate == 1 or kc < SC - 2):
                    drip.popleft()()
            # normalize into a2 staging tiles (bf16); transposes are deferred
            rsA = rs_pool.tile([128, 4], FP, tag="rs", name="rsA")
            rsB = rs_pool.tile([128, 4], FP, tag="rs", name="rsB")
            avAr = avA[:].rearrange("p (q e) -> p q e", e=E1)
            avBr = avB[:].rearrange("p (q e) -> p q e", e=E1)
            a2s = [a2_pool.tile([128, 128], BF, tag="a2", name="a2") for _ in range(4)]
            nc.vector.reciprocal(rsA[:], avAr[:, :, DH : DH + 1])
            for qc in range(4):
                nc.vector.tensor_scalar(
                    a2s[qc][:, 0:64], avA[:, qc * E1 : qc * E1 + DH],
                    rsA[:, qc : qc + 1], None, mybir.AluOpType.mult,
                )
            nc.vector.reciprocal(rsB[:], avBr[:, :, DH : DH + 1])
            for qc in range(4):
                nc.vector.tensor_scalar(
                    a2s[qc][:, 64:128], avB[:, qc * E1 : qc * E1 + DH],
                    rsB[:, qc : qc + 1], None, mybir.AluOpType.mult,
                )

            def tp_flush(m=m, qs=qs, a2s=a2s):
                tp = psum.tile([128, 512], BF, tag="cps", name="tp", bufs=2)
                for qc in range(4):
                    nc.tensor.matmul(
                        tp[:, qc * 128 : (qc + 1) * 128], a2s[qc][:], idn_sb[:],
                        is_transpose=True, skip_group_check=True,
                    )
                nc.vector.tensor_copy(attT[m][:, qs], tp[:])

            # leftover drips emit after the normalize so their copies never
            # delay the av-ring release at the phase boundary
            while drip:
                drip.popleft()()
            return tp_flush

        def c_group(qc, n, wo_all, tag="cps", eng=None):
            def emit():
                cps = psum.tile([128, 512], FP, tag=tag, name="cps", bufs=2)
                for j in range(MC):
                    nc.tensor.matmul(
                        cps[:],
                        attT[j][:, qc * 128 : (qc + 1) * 128],
                        wo_all[:, j * D + n * 512 : j * D + (n + 1) * 512],
                        start=(j == 0), stop=(j == MC - 1),
                    )
                ot = ot_pool.tile([128, 512], FP, tag="ot", name="ot")
                if eng == "act":
                    nc.scalar.copy(ot[:], cps[:])
                else:
                    nc.vector.tensor_copy(ot[:], cps[:])
                nc.sync.dma_start(
                    out[qc * 128 : (qc + 1) * 128, n * 512 : (n + 1) * 512],
                    ot[:],
                )

            return emit

        # ---------- stage A ----------
        with (
            tc.tile_pool(name="xt", bufs=1) as xt_pool,
            tc.tile_pool(name="w", bufs=1) as w_pool,
        ):
            # batched tiles: one DMA each (HWDGE gen is ~650ns per dma_start)
            wkm = [w_pool.tile([128, KC * 128], BF, tag=f"wkm{m}", name=f"wkm{m}") for m in range(MC)]
            wqm = [w_pool.tile([128, KC * 128], BF, tag=f"wqm{m}", name=f"wqm{m}") for m in range(MC)]
            wv_all = w_pool.tile([128, KC * LD], BF, tag="wv", name="wv_all")
            xs = [
                [xt_pool.tile([128, KC * 512], BF, tag=f"x{h}_{j}", name=f"x{h}_{j}") for j in range(2)]
                for h in range(2)
            ]
            wkR = wk.rearrange("(k r) c -> r k c", r=128)
            wqR = wq.rearrange("(k r) c -> r k c", r=128)
            wvR = wv.rearrange("(k r) c -> r k c", r=128)
            xR = xT.rearrange("(k r) q -> r k q", r=128)

            def xv(t):  # [128, (k c)] view
                return t[:].rearrange("r (k c) -> r k c", k=KC)

            # priority order: the tiny bias first (first exp needs it), then
            # inputs of the head chunks, split by contraction half so the
            # first accumulation chain starts early
            nc.sync.dma_start(biasT[:], kbias[:])
            for ks_ in (slice(0, 4), slice(4, 8)):
                nc.sync.dma_start(xv(wkm[0])[:, ks_], wkR[:, ks_, 0:128])
                nc.sync.dma_start(xv(xs[0][0])[:, ks_], xR[:, ks_, 0:512])
                nc.sync.dma_start(xv(wqm[0])[:, ks_], wqR[:, ks_, 0:128])
            # PE warm-up: the cost model halves matmul rate until ~3us of
            # continuous PE activity; spend the DMA-wait head on dummy
            # matmuls over a zeroed tile so real chunks start at full rate
            warm = xt_pool.tile([128, 512], BF, tag="warm", name="warm")
            nc.vector.memset(warm[:], 0.0)
            for _ in range(5):
                wps = psum.tile([128, 512], FP, tag="cps", name="wps", bufs=2)
                nc.tensor.matmul(wps[:], warm[:, 0:128], warm[:], start=True, stop=True)
            nc.sync.dma_start(xv(wv_all), wvR[:, :, :])
            nc.sync.dma_start(xv(xs[0][1]), xR[:, :, 512:1024])
            for h, j in ((1, 0), (1, 1)):
                lo = h * 1024 + j * 512
                nc.sync.dma_start(xv(xs[h][j]), xR[:, :, lo : lo + 512])
            for m in range(1, MC):
                nc.sync.dma_start(xv(wkm[m]), wkR[:, :, m * 128 : (m + 1) * 128])
                nc.sync.dma_start(xv(wqm[m]), wqR[:, :, m * 128 : (m + 1) * 128])
            nc.sync.dma_start(idn_sb[:], idn[:])

            def qk_chunk(half, wtm, dstT, m, q2):
                ps = psum.tile([128, 512], FP, tag="cps", name="psa", bufs=2)
                wts = wtm[m]
                for k in range(KC):
                    nc.tensor.matmul(
                        ps[:],
                        wts[:, k * 128 : (k + 1) * 128],
                        xs[half][q2][:, k * 512 : (k + 1) * 512],
                        start=(k == 0), stop=(k == KC - 1),
                    )
                qlo = half * 1024 + q2 * 512
                nc.vector.tensor_copy(dstT[m][:, qlo : qlo + 512], ps[:])

            def v_chunk(s_idx, m):
                # V columns of head pair m only: the early window needs just
                # pair 0; the rest drips into later phases' slack
                half, sc = s_idx // 8, s_idx % 8
                ps = psum.tile([128, 128], FP, tag="cps", name="psv", bufs=2)
                j, off = sc // 4, (sc % 4) * 128
                for k in range(KC):
                    nc.tensor.matmul(
                        ps[:],
                        xs[half][j][:, k * 512 + off : k * 512 + off + 128],
                        wv_all[:, k * LD + m * 128 : k * LD + (m + 1) * 128],
                        start=(k == 0), stop=(k == KC - 1),
                    )
                src = ps[:].rearrange("p (h e) -> p h e", h=2)
                dst = vt[s_idx][:].rearrange("p (h e) -> p h e", e=E1)[
                    :, 2 * m : 2 * m + 2, 0:DH
                ]
                nc.vector.tensor_copy(dst, src)

            def A(fn, *args):
                return lambda: fn(*args)

            # minimal head: K(m0,h0,q2=0) covers logits kc0..3, Q(m0) first
            # query slice, V(s0..s3) covers the first AV chunks
            qk_chunk(0, wkm, kt, 0, 0)
            qk_chunk(0, wqm, qt, 0, 0)
            for s in range(4):
                v_chunk(s, 0)

            wo_all = w_pool.tile([128, MC * D], BF, tag="wo", name="wo_all")
            nc.sync.dma_start(
                wo_all[:].rearrange("r (j c) -> r j c", j=MC),
                wo.rearrange("(j r) c -> r j c", r=128),
            )

            # (0,0) drip, rate 1: position p pops after logits(p+2); V_s must
            # sit at position <= s-1, K(m0,h*,q2) before logits emission of
            # its key range (kc4@slot2, kc8@slot6, kc12@slot10).
            d00 = [
                A(qk_chunk, 0, wkm, kt, 0, 1),  # keys 512:1024 (kc4+)
                A(v_chunk, 4, 0), A(v_chunk, 5, 0), A(v_chunk, 6, 0),
                A(qk_chunk, 1, wkm, kt, 0, 0),  # keys 1024:1536 (kc8+)
                A(v_chunk, 7, 0), A(v_chunk, 8, 0), A(v_chunk, 9, 0),
                A(qk_chunk, 1, wkm, kt, 0, 1),  # keys 1536:2048 (kc12+)
                A(v_chunk, 10, 0), A(v_chunk, 11, 0), A(v_chunk, 12, 0),
                A(v_chunk, 13, 0), A(v_chunk, 14, 0), A(v_chunk, 15, 0),
                A(qk_chunk, 0, wqm, qt, 0, 1),  # qt1 slice for (0,1)
            ]
            V_ = lambda s, m: A(v_chunk, s, m)

            def C(qti, lo, hi, tag="cps"):  # out-proj groups [lo,hi) of tile qti
                return [c_group(qc, n, wo_all, tag)
                        for g in range(lo, hi)
                        for qc, n in [(4 * qti + g // 2, g % 2)]]

            K_ = lambda m, h, q2: A(qk_chunk, h, wkm, kt, m, q2)
            Q_ = lambda m, h, q2: A(qk_chunk, h, wqm, qt, m, q2)
            # anti-diagonal phase rotation: stage-A chunks spread so nearly
            # every phase stays ACT(exp)-paced; None marks where the previous
            # phase's deferred transpose-flush goes.
            rotation = [
                ((0, 1), 1, [K_(1, 0, 0), Q_(1, 0, 0), None, Q_(0, 1, 0)]
                         + [V_(s, 1) for s in range(0, 12)]),
                ((1, 0), 1, [K_(1, 0, 1), V_(12, 1), K_(1, 1, 0), V_(13, 1),
                             K_(1, 1, 1), V_(14, 1), V_(15, 1), None, Q_(0, 1, 1)]),
                ((0, 2), 2, [Q_(1, 0, 1), K_(2, 0, 0), None]
                         + [V_(s, 2) for s in range(0, 4)]),
                ((1, 1), 2, [Q_(2, 0, 0), Q_(1, 1, 0), None]
                         + [V_(s, 2) for s in range(4, 8)]),
                ((2, 0), 1, [K_(2, 0, 1), V_(8, 2), K_(2, 1, 0), V_(9, 2),
                             K_(2, 1, 1), None]
                         + [V_(s, 2) for s in range(10, 16)]),
                ((0, 3), 2, [Q_(2, 0, 1), None] + [V_(s, 3) for s in range(0, 6)]),
                ((1, 2), 2, [K_(3, 0, 0), Q_(3, 0, 0), None]
                         + [V_(s, 3) for s in range(6, 10)]),
                ((2, 1), 2, [Q_(1, 1, 1), None] + [V_(s, 3) for s in range(10, 14)]),
                ((3, 0), 2, [K_(3, 0, 1), V_(14, 3), K_(3, 1, 0), V_(15, 3),
                             K_(3, 1, 1), None, Q_(2, 1, 0)]),
                ((1, 3), 2, [Q_(3, 0, 1), None] + C(0, 0, 2)),
                ((2, 2), 2, [Q_(2, 1, 1), None] + C(0, 2, 6)),
                ((3, 1), 2, [Q_(3, 1, 0), None] + C(0, 6, 8)),
                ((3, 2), 2, [Q_(3, 1, 1), None] + C(1, 0, 5)),
                ((2, 3), 2, C(1, 5, 8)[:1] + [None] + C(1, 5, 8)[1:]),
                ((3, 3), 2, [None] + C(2, 0, 8)),
            ]

            drip = deque(d00)
            tpf = emit_B(0, 0, drip, rate=1)
            for (m, qti), rate_, items in rotation:
                i = items.index(None)
                drip.extend(items[:i])
                drip.append(tpf)
                drip.extend(items[i + 1 :])
                tpf = emit_B(m, qti, drip, rate=rate_)
            # tail: last transpose-flush + last query tile's output projection,
            # alternating PSUM rings (lg ring is free after the last exp)
            tpf()
            for i in range(8):
                c_group(12 + i // 2, i % 2, wo_all, "lg" if i % 2 else "cps",
                        eng="act" if i % 2 else None)()


_nc = None


def get_nc():
    global _nc
    if _nc is None:
        _nc = build_nc()
    return _nc


def make_in_maps(x, mask, Wq, Wk, Wv, Wo):
    import ml_dtypes

    x = np.asarray(x, dtype=np.float32)
    mask = np.asarray(mask)
    Wq, Wk, Wv, Wo = (np.asarray(w, dtype=np.float32) for w in (Wq, Wk, Wv, Wo))
    in_maps = []
    for c in range(N_CORES):
        b, hg = c // HG, c % HG
        lo, hi = hg * LD, (hg + 1) * LD
        kb = np.where(mask[b], 0.0, NEG_INF).astype(np.float32)
        in_maps.append(
            {
                "xT": np.ascontiguousarray(x[b].T).astype(ml_dtypes.bfloat16),
                "wq": np.ascontiguousarray(Wq[:, lo:hi]).astype(ml_dtypes.bfloat16),
                "wk": np.ascontiguousarray(Wk[:, lo:hi]).astype(ml_dtypes.bfloat16),
                "wv": np.ascontiguousarray(Wv[:, lo:hi]).astype(ml_dtypes.bfloat16),
                "wo": np.ascontiguousarray(Wo[lo:hi, :]).astype(ml_dtypes.bfloat16),
                "kbias": np.ascontiguousarray(kb.reshape(SC, 128).T),
                "idn": np.eye(128, dtype=np.float32).astype(ml_dtypes.bfloat16),
            }
        )
    return in_maps


def kernel(x, mask, Wq, Wk, Wv, Wo):
    nc = get_nc()
    in_maps = make_in_maps(x, mask, Wq, Wk, Wv, Wo)
    res = run_bass_kernel_spmd(nc, in_maps, list(range(N_CORES)))
    outs = np.empty((B, S, D), dtype=np.float32)
    for b in range(B):
        outs[b] = res.results[2 * b]["out"] + res.results[2 * b + 1]["out"]
    return outs


# revision 9
# speedup vs baseline: 1.5000x; 1.0024x over previous
"""Multi-head attention forward on 8 Trainium2 NeuronCores — v3.

Like v2 (AV in [q,d] orientation, bf16 Q/K/attn/Wo, per-partition normalize,
PE transposes) but with pipeline-aware emission for the in-order engines:

- Within a phase (head pair m, query tile qt), logits+exp for key chunk kc+2 are
  emitted BEFORE the AV matmuls of chunk kc, so the ACT engine always has two
  exp instructions in flight when an AV matmul blocks in the PE wait queue.
- The attn transposes of phase i are deferred into phase i+1 (through the cps
  PSUM ring, not the lg ring), so they never delay the next phase's logits.
- Stage-A projection chunks and stage-C output-projection groups drip one per
  key chunk through the cps ring, placed after the logits emission point.
- PSUM: lg [128,1024]x2 (4 banks), avA/avB [128,260]x1 (2), cps [128,512]x2 (2).
"""
import sys

sys.path.insert(0, "/opt/trn_rl_repo")

import numpy as np
from collections import deque

import concourse.bass as bass
import concourse.tile as tile
from concourse import mybir
from concourse.bass_utils import run_bass_kernel_spmd
from concourse.vector_clock import ScopedClock

_wsplit_ctr = [0]


def split_multi_waits(nc):
    """Walrus accepts at most one sync wait per instruction; split extras
    into single-wait nops."""
    for f in nc.m.functions:
        for bb in f.blocks:
            out = []
            changed = False
            for inst in bb.instructions:
                si = inst.sync_info
                waits = list(si.on_wait) if si is not None and si.on_wait else []
                if len(waits) > 1:
                    updates = list(si.on_update) if si.on_update else []
                    for w in waits[1:]:
                        _wsplit_ctr[0] += 1
                        nop = mybir.InstNoOp(
                            name=f"I-wsplit-{_wsplit_ctr[0]}", ins=[], outs=[]
                        )
                        nop.engine = inst.engine
                        nop.sync_info = mybir.SyncInfo(on_wait=[w], on_update=[])
                        out.append(nop)
                    inst.sync_info = mybir.SyncInfo(on_wait=[waits[0]], on_update=updates)
                    changed = True
                out.append(inst)
            if changed:
                bb.instructions = out
    return nc


B, S, D, H, DH = 4, 2048, 1024, 16, 64
HG = 2
LD = D // HG
LH = H // HG
N_CORES = B * HG
SCALE = float(DH) ** -0.5
NEG_INF = -1e30

FP = mybir.dt.float32
FPR = mybir.dt.float32r
BF = mybir.dt.bfloat16

KC = D // 128
MC = LD // 128
SC = S // 128
QT = S // 512
Exp = mybir.ActivationFunctionType.Exp
E1 = DH + 1


def _fr(ap):
    return ap.bitcast(FPR)


class SplitDrainTileContext(tile.TileContext):
    def _drain_and_barrier(self, tick_clock, wait_clock):
        nc = self.nc
        probe = nc.sync.nop()
        wait_clock.add_sem_waits(
            probe.ins, ScopedClock({None: tick_clock.global_clock})
        )
        si = probe.ins.sync_info
        waits = list(si.on_wait) if si is not None and si.on_wait else []
        updates = list(si.on_update) if si is not None and si.on_update else []
        if len(waits) > 1:
            probe.ins.sync_info = mybir.SyncInfo(on_wait=[waits[0]], on_update=updates)
            for w in waits[1:]:
                n2 = nc.sync.nop()
                n2.ins.sync_info = mybir.SyncInfo(on_wait=[w], on_update=[])
        nc.sync.drain()
        nc.all_engine_barrier()
        popped = nc._tile_sem_poison_stack.pop()
        assert popped is self._sem_poison
        nc.clear_and_free_semaphores(list(self.sems.allocated().values()))
        nc.all_engine_barrier()


def build_nc(for_hw=True):
    nc = bass.Bass(trn_type="TRN2")
    xT = nc.dram_tensor("xT", [D, S], BF, kind="ExternalInput").ap()
    wq = nc.dram_tensor("wq", [D, LD], BF, kind="ExternalInput").ap()
    wk = nc.dram_tensor("wk", [D, LD], BF, kind="ExternalInput").ap()
    wv = nc.dram_tensor("wv", [D, LD], BF, kind="ExternalInput").ap()
    wo = nc.dram_tensor("wo", [LD, D], BF, kind="ExternalInput").ap()
    kbias = nc.dram_tensor("kbias", [128, SC], FP, kind="ExternalInput").ap()
    idn = nc.dram_tensor("idn", [128, 128], BF, kind="ExternalInput").ap()
    out = nc.dram_tensor("out", [S, D], FP, kind="ExternalOutput").ap()

    with SplitDrainTileContext(nc) as tc:
        _body(tc, xT, wq, wk, wv, wo, kbias, idn, out)
    if for_hw:
        split_multi_waits(nc)
    return nc


def _body(tc, xT, wq, wk, wv, wo, kbias, idn, out):
    nc = tc.nc
    with (
        tc.tile_pool(name="pers", bufs=1) as pers,
        tc.tile_pool(name="pt", bufs=6) as pt_pool,
        tc.tile_pool(name="a2", bufs=8) as a2_pool,
        tc.tile_pool(name="rs", bufs=4) as rs_pool,
        tc.tile_pool(name="ot", bufs=6) as ot_pool,
        tc.tile_pool(name="psmm", bufs=1, space="PSUM") as psum,
    ):
        qt = [pers.tile([128, S], BF, tag=f"qt{m}", name=f"qt{m}") for m in range(MC)]
        kt = [pers.tile([128, S], BF, tag=f"kt{m}", name=f"kt{m}") for m in range(MC)]
        vt = [pers.tile([128, LH * E1], BF, tag=f"v{s}", name=f"v{s}") for s in range(SC)]
        attT = [pers.tile([128, S], BF, tag=f"at{m}", name=f"at{m}") for m in range(MC)]
        biasT = pers.tile([128, SC], FP, tag="biasT")
        idn_sb = pers.tile([128, 128], BF, tag="idn")

        for s in range(SC):
            dst = vt[s][:].rearrange("p (h e) -> p h e", e=E1)[:, :, DH : DH + 1]
            nc.vector.memset(dst, 1.0)

        # ---------- stage B phase emitter ----------
        def emit_B(m, qti, drip, rate=2, tail=None):
            """One attention phase. Emits logits/exp two key-chunks ahead of
            the AV matmuls; pops drip closures after the logits point of
            every `rate`-th key chunk."""
            hA, hB = 2 * m, 2 * m + 1
            qs = slice(qti * 512, (qti + 1) * 512)
            avA = psum.tile([128, 4 * E1], FP, tag="avA", name="avA", bufs=1)
            avB = psum.tile([128, 4 * E1], FP, tag="avB", name="avB", bufs=1)
            pts = {}

            def logits(kc):
                ks = slice(kc * 128, (kc + 1) * 128)
                lg = psum.tile([128, 1024], FP, tag="lg", name="lg", bufs=2)
                nc.tensor.matmul(
                    lg[:, 0:512], kt[m][0:64, ks], qt[m][0:64, qs],
                    start=True, stop=True,
                )
                nc.tensor.matmul(
                    lg[:, 512:1024], kt[m][64:128, ks], qt[m][64:128, qs],
                    start=True, stop=True,
                )
                pt = pt_pool.tile([128, 1024], BF, tag="pt", name="pt")
                nc.scalar.activation(
                    pt[:], lg[:], Exp, bias=biasT[:, kc : kc + 1], scale=SCALE
                )
                pts[kc] = pt

            logits(0)
            logits(1)
            for kc in range(SC):
                pt = pts.pop(kc)
                for qc in range(4):
                    # start=True only on the first slice: the PSUM zero-region
                    # "pending zero" marking spans the whole bank, so later
                    # start=True calls would wipe sibling slices' first chunk
                    nc.tensor.matmul(
                        avA[:, qc * E1 : (qc + 1) * E1],
                        pt[:, qc * 128 : (qc + 1) * 128],
                        vt[kc][:, hA * E1 : (hA + 1) * E1],
                        start=(kc == 0 and qc == 0), stop=(kc == SC - 1),
                        skip_group_check=True,
                    )
                    nc.tensor.matmul(
                        avB[:, qc * E1 : (qc + 1) * E1],
                        pt[:, 512 + qc * 128 : 512 + (qc + 1) * 128],
                        vt[kc][:, hB * E1 : (hB + 1) * E1],
                        start=(kc == 0 and qc == 0), stop=(kc == SC - 1),
                        skip_group_check=True,
                    )
                if kc + 2 < SC:
                    logits(kc + 2)
                if drip and kc % rate == rate - 1 and (rate == 1 or kc < SC - 2):
                    drip.popleft()()
            # normalize into a2 staging tiles (bf16); transposes are deferred
            rsA = rs_pool.tile([128, 4], FP, tag="rs", name="rsA")
            rsB = rs_pool.tile([128, 4], FP, tag="rs", name="rsB")
            avAr = avA[:].rearrange("p (q e) -> p q e", e=E1)
            avBr = avB[:].rearrange("p (q e) -> p q e", e=E1)
            a2s = [a2_pool.tile([128, 128], BF, tag="a2", name="a2") for _ in range(4)]
            if tail is not None:
                # last phase: pipeline per query-chunk so each chunk's output
                # projection starts while the next chunk still normalizes
                while drip:
                    drip.popleft()()
                nc.vector.reciprocal(rsA[:], avAr[:, :, DH : DH + 1])
                nc.vector.reciprocal(rsB[:], avBr[:, :, DH : DH + 1])
                tp = psum.tile([128, 512], BF, tag="lg", name="tp", bufs=2)
                for qc in range(4):
                    nc.vector.tensor_scalar(
                        a2s[qc][:, 0:64], avA[:, qc * E1 : qc * E1 + DH],
                        rsA[:, qc : qc + 1], None, mybir.AluOpType.mult,
                    )
                    nc.vector.tensor_scalar(
                        a2s[qc][:, 64:128], avB[:, qc * E1 : qc * E1 + DH],
                        rsB[:, qc : qc + 1], None, mybir.AluOpType.mult,
                    )
                    nc.tensor.matmul(
                        tp[:, qc * 128 : (qc + 1) * 128], a2s[qc][:], idn_sb[:],
                        is_transpose=True, skip_group_check=True,
                    )
                    lo = qti * 512 + qc * 128
                    nc.vector.tensor_copy(
                        attT[m][:, lo : lo + 128],
                        tp[:, qc * 128 : (qc + 1) * 128],
                    )
                    tail(4 * qti + qc)
                return None
            nc.vector.reciprocal(rsA[:], avAr[:, :, DH : DH + 1])
            for qc in range(4):
                nc.vector.tensor_scalar(
                    a2s[qc][:, 0:64], avA[:, qc * E1 : qc * E1 + DH],
                    rsA[:, qc : qc + 1], None, mybir.AluOpType.mult,
                )
            nc.vector.reciprocal(rsB[:], avBr[:, :, DH : DH + 1])
            for qc in range(4):
                nc.vector.tensor_scalar(
                    a2s[qc][:, 64:128], avB[:, qc * E1 : qc * E1 + DH],
                    rsB[:, qc : qc + 1], None, mybir.AluOpType.mult,
                )

            def tp_flush(m=m, qs=qs, a2s=a2s):
                tp = psum.tile([128, 512], BF, tag="cps", name="tp", bufs=2)
                for qc in range(4):
                    nc.tensor.matmul(
                        tp[:, qc * 128 : (qc + 1) * 128], a2s[qc][:], idn_sb[:],
                        is_transpose=True, skip_group_check=True,
                    )
                nc.vector.tensor_copy(attT[m][:, qs], tp[:])

            # leftover drips emit after the normalize so their copies never
            # delay the av-ring release at the phase boundary
            while drip:
                drip.popleft()()
            return tp_flush

        def c_group(qc, n, wo_all, tag="cps", eng=None):
            def emit():
                cps = psum.tile([128, 512], FP, tag=tag, name="cps", bufs=2)
                for j in range(MC):
                    nc.tensor.matmul(
                        cps[:],
                        attT[j][:, qc * 128 : (qc + 1) * 128],
                        wo_all[:, j * D + n * 512 : j * D + (n + 1) * 512],
                        start=(j == 0), stop=(j == MC - 1),
                    )
                ot = ot_pool.tile([128, 512], FP, tag="ot", name="ot")
                if eng == "act":
                    nc.scalar.copy(ot[:], cps[:])
                else:
                    nc.vector.tensor_copy(ot[:], cps[:])
                nc.sync.dma_start(
                    out[qc * 128 : (qc + 1) * 128, n * 512 : (n + 1) * 512],
                    ot[:],
                )

            return emit

        # ---------- stage A ----------
        with (
            tc.tile_pool(name="xt", bufs=1) as xt_pool,
            tc.tile_pool(name="w", bufs=1) as w_pool,
        ):
            # batched tiles: one DMA each (HWDGE gen is ~650ns per dma_start)
            wkm = [w_pool.tile([128, KC * 128], BF, tag=f"wkm{m}", name=f"wkm{m}") for m in range(MC)]
            wqm = [w_pool.tile([128, KC * 128], BF, tag=f"wqm{m}", name=f"wqm{m}") for m in range(MC)]
            wv_all = w_pool.tile([128, KC * LD], BF, tag="wv", name="wv_all")
            xs = [
                [xt_pool.tile([128, KC * 512], BF, tag=f"x{h}_{j}", name=f"x{h}_{j}") for j in range(2)]
                for h in range(2)
            ]
            wkR = wk.rearrange("(k r) c -> r k c", r=128)
            wqR = wq.rearrange("(k r) c -> r k c", r=128)
            wvR = wv.rearrange("(k r) c -> r k c", r=128)
            xR = xT.rearrange("(k r) q -> r k q", r=128)

            def xv(t):  # [128, (k c)] view
                return t[:].rearrange("r (k c) -> r k c", k=KC)

            # priority order: the tiny bias first (first exp needs it), then
            # inputs of the head chunks, split by contraction half so the
            # first accumulation chain starts early
            nc.sync.dma_start(biasT[:], kbias[:])
            for ks_ in (slice(0, 4), slice(4, 8)):
                nc.sync.dma_start(xv(wkm[0])[:, ks_], wkR[:, ks_, 0:128])
                nc.sync.dma_start(xv(xs[0][0])[:, ks_], xR[:, ks_, 0:512])
                nc.sync.dma_start(xv(wqm[0])[:, ks_], wqR[:, ks_, 0:128])
            # PE warm-up: the cost model halves matmul rate until ~3us of
            # continuous PE activity; spend the DMA-wait head on dummy
            # matmuls over a zeroed tile so real chunks start at full rate
            warm = xt_pool.tile([128, 512], BF, tag="warm", name="warm")
            nc.vector.memset(warm[:], 0.0)
            for _ in range(5):
                wps = psum.tile([128, 512], FP, tag="cps", name="wps", bufs=2)
                nc.tensor.matmul(wps[:], warm[:, 0:128], warm[:], start=True, stop=True)
            nc.sync.dma_start(xv(wv_all), wvR[:, :, :])
            nc.sync.dma_start(xv(xs[0][1]), xR[:, :, 512:1024])
            for h, j in ((1, 0), (1, 1)):
                lo = h * 1024 + j * 512
                nc.sync.dma_start(xv(xs[h][j]), xR[:, :, lo : lo + 512])
            for m in range(1, MC):
                nc.sync.dma_start(xv(wkm[m]), wkR[:, :, m * 128 : (m + 1) * 128])
                nc.sync.dma_start(xv(wqm[m]), wqR[:, :, m * 128 : (m + 1) * 128])
            nc.sync.dma_start(idn_sb[:], idn[:])

            def qk_chunk(half, wtm, dstT, m, q2):
                ps = psum.tile([128, 512], FP, tag="cps", name="psa", bufs=2)
                wts = wtm[m]
                for k in range(KC):
                    nc.tensor.matmul(
                        ps[:],
                        wts[:, k * 128 : (k + 1) * 128],
                        xs[half][q2][:, k * 512 : (k + 1) * 512],
                        start=(k == 0), stop=(k == KC - 1),
                    )
                qlo = half * 1024 + q2 * 512
                nc.vector.tensor_copy(dstT[m][:, qlo : qlo + 512], ps[:])

            def v_chunk(s_idx, m):
                # V columns of head pair m only: the early window needs just
                # pair 0; the rest drips into later phases' slack
                half, sc = s_idx // 8, s_idx % 8
                ps = psum.tile([128, 128], FP, tag="cps", name="psv", bufs=2)
                j, off = sc // 4, (sc % 4) * 128
                for k in range(KC):
                    nc.tensor.matmul(
                        ps[:],
                        xs[half][j][:, k * 512 + off : k * 512 + off + 128],
                        wv_all[:, k * LD + m * 128 : k * LD + (m + 1) * 128],
                        start=(k == 0), stop=(k == KC - 1),
                    )
                src = ps[:].rearrange("p (h e) -> p h e", h=2)
                dst = vt[s_idx][:].rearrange("p (h e) -> p h e", e=E1)[
                    :, 2 * m : 2 * m + 2, 0:DH
                ]
                nc.vector.tensor_copy(dst, src)

            def A(fn, *args):
                return lambda: fn(*args)

            # minimal head: K(m0,h0,q2=0) covers logits kc0..3, Q(m0) first
            # query slice, V(s0..s3) covers the first AV chunks
            qk_chunk(0, wkm, kt, 0, 0)
            qk_chunk(0, wqm, qt, 0, 0)
            for s in range(4):
                v_chunk(s, 0)

            wo_all = w_pool.tile([128, MC * D], BF, tag="wo", name="wo_all")
            nc.sync.dma_start(
                wo_all[:].rearrange("r (j c) -> r j c", j=MC),
                wo.rearrange("(j r) c -> r j c", r=128),
            )

            # (0,0) drip, rate 1: position p pops after logits(p+2); V_s must
            # sit at position <= s-1, K(m0,h*,q2) before logits emission of
            # its key range (kc4@slot2, kc8@slot6, kc12@slot10).
            d00 = [
                A(qk_chunk, 0, wkm, kt, 0, 1),  # keys 512:1024 (kc4+)
                A(v_chunk, 4, 0), A(v_chunk, 5, 0), A(v_chunk, 6, 0),
                A(qk_chunk, 1, wkm, kt, 0, 0),  # keys 1024:1536 (kc8+)
                A(v_chunk, 7, 0), A(v_chunk, 8, 0), A(v_chunk, 9, 0),
                A(qk_chunk, 1, wkm, kt, 0, 1),  # keys 1536:2048 (kc12+)
                A(v_chunk, 10, 0), A(v_chunk, 11, 0), A(v_chunk, 12, 0),
                A(v_chunk, 13, 0), A(v_chunk, 14, 0), A(v_chunk, 15, 0),
                A(qk_chunk, 0, wqm, qt, 0, 1),  # qt1 slice for (0,1)
            ]
            V_ = lambda s, m: A(v_chunk, s, m)

            def C(qti, lo, hi, tag="cps"):  # out-proj groups [lo,hi) of tile qti
                return [c_group(qc, n, wo_all, tag)
                        for g in range(lo, hi)
                        for qc, n in [(4 * qti + g // 2, g % 2)]]

            K_ = lambda m, h, q2: A(qk_chunk, h, wkm, kt, m, q2)
            Q_ = lambda m, h, q2: A(qk_chunk, h, wqm, qt, m, q2)
            # anti-diagonal phase rotation: stage-A chunks spread so nearly
            # every phase stays ACT(exp)-paced; None marks where the previous
            # phase's deferred transpose-flush goes.
            rotation = [
                ((0, 1), 1, [K_(1, 0, 0), Q_(1, 0, 0), None, Q_(0, 1, 0)]
                         + [V_(s, 1) for s in range(0, 12)]),
                ((1, 0), 1, [K_(1, 0, 1), V_(12, 1), K_(1, 1, 0), V_(13, 1),
                             K_(1, 1, 1), V_(14, 1), V_(15, 1), None, Q_(0, 1, 1)]),
                ((0, 2), 2, [Q_(1, 0, 1), K_(2, 0, 0), None]
                         + [V_(s, 2) for s in range(0, 4)]),
                ((1, 1), 2, [Q_(2, 0, 0), Q_(1, 1, 0), None]
                         + [V_(s, 2) for s in range(4, 8)]),
                ((2, 0), 1, [K_(2, 0, 1), V_(8, 2), K_(2, 1, 0), V_(9, 2),
                             K_(2, 1, 1), None]
                         + [V_(s, 2) for s in range(10, 16)]),
                ((0, 3), 2, [Q_(2, 0, 1), None] + [V_(s, 3) for s in range(0, 6)]),
                ((1, 2), 2, [K_(3, 0, 0), Q_(3, 0, 0), None]
                         + [V_(s, 3) for s in range(6, 10)]),
                ((2, 1), 2, [Q_(1, 1, 1), None] + [V_(s, 3) for s in range(10, 14)]),
                ((3, 0), 2, [K_(3, 0, 1), V_(14, 3), K_(3, 1, 0), V_(15, 3),
                             K_(3, 1, 1), None, Q_(2, 1, 0)]),
                ((1, 3), 2, [Q_(3, 0, 1), None] + C(0, 0, 2)),
                ((2, 2), 2, [Q_(2, 1, 1), None] + C(0, 2, 6)),
                ((3, 1), 2, [Q_(3, 1, 0), None] + C(0, 6, 8)),
                ((3, 2), 2, [Q_(3, 1, 1), None] + C(1, 0, 5)),
                ((2, 3), 2, C(1, 5, 8)[:1] + [None] + C(1, 5, 8)[1:] + C(2, 0, 2)),
                ((3, 3), 2, [None] + C(2, 2, 8)),
            ]

            drip = deque(d00)
            tpf = emit_B(0, 0, drip, rate=1)
            for (m, qti), rate_, items in rotation:
                i = items.index(None)
                drip.extend(items[:i])
                drip.append(tpf)
                drip.extend(items[i + 1 :])
                if (m, qti) == (3, 3):
                    def tail_c(qc):
                        for n in range(2):
                            c_group(qc, n, wo_all, "cps",
                                    eng="act" if n else None)()

                    emit_B(m, qti, drip, rate=rate_, tail=tail_c)
                else:
                    tpf = emit_B(m, qti, drip, rate=rate_)


_nc = None


def get_nc():
    global _nc
    if _nc is None:
        _nc = build_nc()
    return _nc


def make_in_maps(x, mask, Wq, Wk, Wv, Wo):
    import ml_dtypes

    x = np.asarray(x, dtype=np.float32)
    mask = np.asarray(mask)
    Wq, Wk, Wv, Wo = (np.asarray(w, dtype=np.float32) for w in (Wq, Wk, Wv, Wo))
    in_maps = []
    for c in range(N_CORES):
        b, hg = c // HG, c % HG
        lo, hi = hg * LD, (hg + 1) * LD
        kb = np.where(mask[b], 0.0, NEG_INF).astype(np.float32)
        in_maps.append(
            {
                "xT": np.ascontiguousarray(x[b].T).astype(ml_dtypes.bfloat16),
                "wq": np.ascontiguousarray(Wq[:, lo:hi]).astype(ml_dtypes.bfloat16),
                "wk": np.ascontiguousarray(Wk[:, lo:hi]).astype(ml_dtypes.bfloat16),
                "wv": np.ascontiguousarray(Wv[:, lo:hi]).astype(ml_dtypes.bfloat16),
                "wo": np.ascontiguousarray(Wo[lo:hi, :]).astype(ml_dtypes.bfloat16),
                "kbias": np.ascontiguousarray(kb.reshape(SC, 128).T),
                "idn": np.eye(128, dtype=np.float32).astype(ml_dtypes.bfloat16),
            }
        )
    return in_maps


def kernel(x, mask, Wq, Wk, Wv, Wo):
    nc = get_nc()
    in_maps = make_in_maps(x, mask, Wq, Wk, Wv, Wo)
    res = run_bass_kernel_spmd(nc, in_maps, list(range(N_CORES)))
    outs = np.empty((B, S, D), dtype=np.float32)
    for b in range(B):
        outs[b] = res.results[2 * b]["out"] + res.results[2 * b + 1]["out"]
    return outs
